# revision 11
# baseline (speedup 1.0000x reference)
"""DeepseekV3 decoder layer (MLA + SwiGLU MLP), T=2048 prefill, fp32 I/O.

Sharding: sequence-parallel striped - core c owns token rows c::8 (256 rows),
so all 8 cores run one identical SPMD program with balanced causal work.
The KV latent path (all 2048 tokens) is replicated on every core; outputs
are disjoint row sets concatenated on host.

v2 redesign vs v1:
- Causal tightening: with striped rows, key tile kt is attended only by
  query rows >= 16*kt on EVERY core, so score/attn matmuls shrink their
  free dim per key tile (2176 vs 3072 q-cols of work) and masking reduces
  to one [128,16] triangle applied to the first 16 active columns.
- Attention computed fully in transposed layouts: qnT/qT_lat directly from
  qcT (no per-head transposes), o_latent^T accumulated directly via
  lhsT=c_hat tiles, denominator via an appended ones-column in c_hat,
  1/den applied to o_vT via a K=1 broadcast matmul.
- SCALE folded into q_b weights on host; LN weights folded as in v1.
- MLP/o_proj weights streamed on the gpsimd (SWDGE) queue so the big
  weight stream never head-of-line-blocks latency-critical sync-queue
  loads; gate+up fused to 1MB DMAs, down proj 1MB (2 tiles per DMA).
"""

import numpy as np
import ml_dtypes

bfloat16 = ml_dtypes.bfloat16

T = 2048
H = 2048
NH = 16
QLR = 1536
KVLR = 512
DN = 128
DR = 64
DV = 128
INTER = 10944
NCORES = 8
RPC = T // NCORES          # 256 query rows per core
NQT = RPC // 128           # 2
NTT = T // 128             # 16
NFC = H // 128             # 16
NRC = QLR // 128           # 12
NKV = KVLR // 128          # 4
NIT = 86
IPAD = NIT * 128
EPS = 1e-6
SCALE = (DN + DR) ** -0.5
THETA = 10000.0
CAUG = 520                 # c_hat free width: 512 latent + 1 ones + 7 pad

_CACHE = {}


def _build_module():
    import os
    MAXPH = int(os.environ.get("KERNEL_MAXPH", "9"))
    import concourse.bass as bass
    import concourse.tile as tile
    from concourse import bacc, mybir

    f32 = mybir.dt.float32
    bf16 = mybir.dt.bfloat16
    AF = mybir.ActivationFunctionType
    ALU = mybir.AluOpType

    nc = bacc.Bacc("TRN2", target_bir_lowering=False, debug=False,
                   enable_asserts=False, num_devices=NCORES)

    def inp(name, shape, dt):
        return nc.dram_tensor(name, list(shape), dt, kind="ExternalInput").ap()

    # per-core inputs
    x_rows = inp("x_rows", [NQT, 128, H], f32)
    xTc = inp("xTc", [NFC, 128, RPC], bf16)
    cosq16 = inp("cosq16", [NQT, 128, NH * DR // 2], bf16)
    sinq16 = inp("sinq16", [NQT, 128, NH * DR // 2], bf16)
    mask16 = inp("mask16", [128, 16], bf16)
    # replicated inputs
    xstat = inp("xstat", [NTT, 128, H], bf16)
    xT_blk = inp("xT_blk", [NTT, 128, NFC, 128], bf16)
    qa_blk = inp("qa_blk", [NFC, 128, QLR], bf16)
    qbn_blk = inp("qbn_blk", [NH, 128, NRC * DN], bf16)
    qbr_blk = inp("qbr_blk", [NRC, 128, NH * DR], bf16)
    kva_blk = inp("kva_blk", [NFC, 128, KVLR + DR], bf16)
    wuk = inp("wuk", [NH, 128, NKV, 128], bf16)
    wuv = inp("wuv", [NH, 128, NKV, DV], bf16)
    ow_blk = inp("ow_blk", [NH, 128, H], bf16)
    gu_blk = inp("gu_blk", [NIT, 128, 2 * NFC * 128], bf16)
    dw_blk = inp("dw_blk", [NIT // 2, 128, 2 * H], bf16)
    cosk = inp("cosk", [128, NTT, DR // 2], f32)
    sink = inp("sink", [128, NTT, DR // 2], f32)
    eye = inp("eye", [128, 128], bf16)
    ones_row = inp("ones_row", [1, 128], bf16)

    out_rows = nc.dram_tensor("out_rows", [NQT, 128, H], f32,
                              kind="ExternalOutput").ap()

    from contextlib import ExitStack
    with tile.TileContext(nc) as tc, ExitStack() as ctx:
        persist = ctx.enter_context(tc.tile_pool(name="persist", bufs=1))

        def pt(shape, dt, tag):
            return persist.tile(list(shape), dt, tag=tag, name=tag)

        eps_sb = pt([128, 1], f32, "eps")
        nc.vector.memset(eps_sb[:], EPS)
        eye_sb = pt([128, 128], bf16, "eye")
        nc.sync.dma_start(out=eye_sb[:], in_=eye[:])
        ones_sb = pt([1, 128], bf16, "ones")
        nc.sync.dma_start(out=ones_sb[:], in_=ones_row[:])
        mask_sb = pt([128, 16], bf16, "mask")
        nc.sync.dma_start(out=mask_sb[:], in_=mask16[:])
        x_rows_sb = pt([128, NQT, H], f32, "x_rows")
        for qt in range(NQT):
            nc.sync.dma_start(out=x_rows_sb[:, qt, :], in_=x_rows[qt])
        cq_sb = pt([128, NQT, NH * DR // 2], bf16, "cq")
        sq_sb = pt([128, NQT, NH * DR // 2], bf16, "sq")
        for qt in range(NQT):
            nc.sync.dma_start(out=cq_sb[:, qt, :], in_=cosq16[qt])
            nc.sync.dma_start(out=sq_sb[:, qt, :], in_=sinq16[qt])

        rstd_all = pt([128, NTT], f32, "rstd_all")
        c_hat = pt([128, NTT, CAUG], bf16, "c_hat")
        # ones column + zero pad for the denominator trick
        nc.vector.memset(c_hat[:, :, 512:513], 1.0)
        nc.vector.memset(c_hat[:, :, 513:CAUG], 0.0)
        kT_lat = pt([128, NKV, T], bf16, "kT_lat")
        kT_rope = pt([64, T], bf16, "kT_rope")
        qcT = pt([128, NRC, RPC], bf16, "qcT")
        qropeT = pt([64, NH, RPC], bf16, "qropeT")
        o_vT = pt([128, NH, RPC], bf16, "o_vT")
        hnT = pt([128, NFC, RPC], bf16, "hnT")
        act_all = pt([128, NIT, RPC], bf16, "act_all")

        # =============== phase B1: row stats + q_a GEMM ===============
        with tc.tile_pool(name="pb", bufs=2) as pb, \
             tc.tile_pool(name="pbs", bufs=1) as pbs, \
             tc.tile_pool(name="pbd", bufs=2) as pbd, \
             tc.tile_pool(name="pbps", bufs=2, space="PSUM") as pbps, \
             tc.tile_pool(name="pbtp", bufs=2, space="PSUM") as pbtp:
            rstd_rows = pbs.tile([128, NQT], f32, name="rstd_rows")
            ssq_r = pbs.tile([128, NQT], f32, name="ssq_r")
            for qt in range(NQT):
                scrap = pbd.tile([128, H], bf16, tag="scrapq", name="scrapq")
                nc.vector.scalar_tensor_tensor(
                    scrap[:], x_rows_sb[:, qt, :], 1.0, x_rows_sb[:, qt, :],
                    ALU.bypass, ALU.mult, accum_out=ssq_r[:, qt:qt + 1])
            nc.scalar.activation(rstd_rows[:], ssq_r[:], AF.Ln,
                                 bias=eps_sb[:], scale=1.0 / H)
            nc.scalar.activation(rstd_rows[:], rstd_rows[:], AF.Exp, scale=-0.5)

            xTc_sb = pbs.tile([128, NFC, RPC], bf16, name="xTc_sb")
            for fc in range(NFC):
                nc.sync.dma_start(out=xTc_sb[:, fc, :], in_=xTc[fc])
            qa_ps = [pbps.tile([128, QLR], f32, tag="qaps", name="qaps")
                     for _ in range(NQT)]
            for fc in range(NFC):
                qaw = pb.tile([128, QLR], bf16, tag="qaw", name="qaw")
                nc.sync.dma_start(out=qaw[:], in_=qa_blk[fc])
                for qt in range(NQT):
                    for nn in range(QLR // 512):
                        nc.tensor.matmul(
                            qa_ps[qt][:, nn * 512:(nn + 1) * 512],
                            xTc_sb[:, fc, qt * 128:(qt + 1) * 128],
                            qaw[:, nn * 512:(nn + 1) * 512],
                            start=(fc == 0), stop=(fc == NFC - 1))
            qc = pbs.tile([128, NQT, QLR], bf16, name="qc")
            ssq_q = pbs.tile([128, NQT], f32, name="ssq_q")
            for qt in range(NQT):
                scr = pbd.tile([128, QLR], bf16, tag="scrq2", name="scrq2")
                nc.scalar.activation(scr[:], qa_ps[qt][:], AF.Square,
                                     accum_out=ssq_q[:, qt:qt + 1])
            sq = pbs.tile([128, NQT], f32, name="sqn")
            nc.vector.tensor_mul(sq[:], rstd_rows[:], rstd_rows[:])
            nc.vector.tensor_mul(sq[:], sq[:], ssq_q[:])
            nc.scalar.activation(sq[:], sq[:], AF.Ln, bias=eps_sb[:],
                                 scale=1.0 / QLR)
            nc.scalar.activation(sq[:], sq[:], AF.Exp, scale=-0.5)
            nc.vector.tensor_mul(sq[:], rstd_rows[:], sq[:])
            for qt in range(NQT):
                nc.vector.tensor_scalar_mul(qc[:, qt, :], qa_ps[qt][:],
                                            sq[:, qt:qt + 1])
            for qt in range(NQT):
                for rc in range(NRC):
                    tp = pbtp.tile([128, 128], bf16, tag="tp", name="tp")
                    nc.tensor.transpose(tp[:], qc[:, qt, rc * 128:(rc + 1) * 128],
                                        eye_sb[:])
                    nc.any.tensor_copy(qcT[:, rc, qt * 128:(qt + 1) * 128],
                                       tp[:])

        # =============== phase B2: rope-q for all heads ===============
        if MAXPH >= 1:
            with tc.tile_pool(name="pr", bufs=2) as pr, \
                 tc.tile_pool(name="prs", bufs=1) as prs, \
                 tc.tile_pool(name="prd", bufs=2) as prd, \
                 tc.tile_pool(name="prps", bufs=2, space="PSUM") as prps, \
                 tc.tile_pool(name="prtp", bufs=2, space="PSUM") as prtp:
                qbr_sb = prs.tile([128, NRC, NH * DR], bf16, name="qbr_sb")
                for rc in range(NRC):
                    nc.sync.dma_start(out=qbr_sb[:, rc, :], in_=qbr_blk[rc])
                qr_sb = prs.tile([128, NQT, NH * DR], bf16, name="qr_sb")
                ND = NH * DR
                for qt in range(NQT):
                    rps = prps.tile([128, ND], f32, tag="rps", name="rps")
                    for rc in range(NRC):
                        for half in range(2):
                            nc.tensor.matmul(
                                rps[:, half * 512:(half + 1) * 512],
                                qcT[:, rc, qt * 128:(qt + 1) * 128],
                                qbr_sb[:, rc, half * 512:(half + 1) * 512],
                                start=(rc == 0), stop=(rc == NRC - 1))
                    # rotate pairs: flat col h*64+2i <-> (head h, pair i);
                    # cq/sq are packed h-major [128, NH*32] to match
                    x1 = rps[:, 0:ND:2]
                    x2 = rps[:, 1:ND:2]
                    cs = cq_sb[:, qt, :]
                    sn = sq_sb[:, qt, :]
                    ta = prd.tile([128, ND // 2], f32, tag="ta", name="ta")
                    tb = prd.tile([128, ND // 2], f32, tag="tb", name="tb")
                    nc.vector.tensor_mul(ta[:], x1, cs)
                    nc.vector.tensor_mul(tb[:], x2, sn)
                    nc.vector.tensor_sub(qr_sb[:, qt, 0:ND:2], ta[:], tb[:])
                    nc.vector.tensor_mul(ta[:], x2, cs)
                    nc.vector.tensor_mul(tb[:], x1, sn)
                    nc.vector.tensor_add(qr_sb[:, qt, 1:ND:2], ta[:], tb[:])
                for qt in range(NQT):
                    for h in range(NH):
                        tp = prtp.tile([128, 128], bf16, tag="tp", name="tp")
                        nc.tensor.transpose(
                            tp[0:64, :],
                            qr_sb[:, qt, h * DR:(h + 1) * DR], eye_sb[:])
                        nc.any.tensor_copy(
                            qropeT[:, h, qt * 128:(qt + 1) * 128], tp[0:64, :])

        # =============== phase 0: all-token stats + kv path ===============
        if MAXPH >= 2:
            with tc.tile_pool(name="p0", bufs=2) as p0, \
                 tc.tile_pool(name="p0x", bufs=2) as p0x, \
                 tc.tile_pool(name="p0w", bufs=1) as p0w, \
                 tc.tile_pool(name="p0s", bufs=1) as p0s, \
                 tc.tile_pool(name="p0d", bufs=2) as p0d, \
                 tc.tile_pool(name="p0ps", bufs=2, space="PSUM") as p0ps, \
                 tc.tile_pool(name="p0tp", bufs=2, space="PSUM") as p0tp:
                cosk_sb = p0s.tile([128, NTT, DR // 2], f32, name="cosk_sb")
                nc.sync.dma_start(out=cosk_sb[:], in_=cosk[:])
                sink_sb = p0s.tile([128, NTT, DR // 2], f32, name="sink_sb")
                nc.sync.dma_start(out=sink_sb[:], in_=sink[:])
                ssq_all = p0s.tile([128, NTT], f32, name="ssq_all")
                for tt in range(NTT):
                    xs = p0x.tile([128, H], bf16, tag="xs", name="xs")
                    nc.sync.dma_start(out=xs[:], in_=xstat[tt])
                    scrap = p0d.tile([128, H], bf16, tag="scrap", name="scrap")
                    nc.vector.scalar_tensor_tensor(
                        scrap[:], xs[:], 1.0, xs[:], ALU.bypass, ALU.mult,
                        accum_out=ssq_all[:, tt:tt + 1])
                nc.scalar.activation(rstd_all[:], ssq_all[:], AF.Ln,
                                     bias=eps_sb[:], scale=1.0 / H)
                nc.scalar.activation(rstd_all[:], rstd_all[:], AF.Exp,
                                     scale=-0.5)

                kvw = p0w.tile([128, NFC, KVLR + DR], bf16, name="kvw")
                for fc in range(NFC):
                    nc.sync.dma_start(out=kvw[:, fc, :], in_=kva_blk[fc])
                kr_all = p0s.tile([128, NTT, DR], bf16, name="kr_all")
                ssq_kv = p0s.tile([128, NTT], f32, name="ssq_kv")
                s_ck = p0s.tile([128, NTT], f32, name="s_ck")
                t1 = p0s.tile([128, NTT], f32, name="t1")
                for tt in range(NTT):
                    xt = p0.tile([128, NFC, 128], bf16, tag="xt", name="xt")
                    nc.sync.dma_start(out=xt[:], in_=xT_blk[tt])
                    ps = p0ps.tile([128, KVLR + DR], f32, tag="kvps",
                                   name="kvps")
                    for fc in range(NFC):
                        nc.tensor.matmul(ps[:, 0:512], xt[:, fc, :],
                                         kvw[:, fc, 0:512],
                                         start=(fc == 0), stop=(fc == NFC - 1))
                        nc.tensor.matmul(ps[:, 512:576], xt[:, fc, :],
                                         kvw[:, fc, 512:576],
                                         start=(fc == 0), stop=(fc == NFC - 1))
                    scr2 = p0d.tile([128, KVLR], bf16, tag="scr2", name="scr2")
                    nc.scalar.activation(scr2[:], ps[:, 0:512], AF.Square,
                                         accum_out=ssq_kv[:, tt:tt + 1])
                    # s_ck[tt] = rstd * rsqrt(mean((rstd*kv)^2)+eps)
                    nc.vector.tensor_mul(t1[:, tt:tt + 1],
                                         rstd_all[:, tt:tt + 1],
                                         rstd_all[:, tt:tt + 1])
                    nc.vector.tensor_mul(t1[:, tt:tt + 1], t1[:, tt:tt + 1],
                                         ssq_kv[:, tt:tt + 1])
                    nc.scalar.activation(t1[:, tt:tt + 1], t1[:, tt:tt + 1],
                                         AF.Ln, bias=eps_sb[:],
                                         scale=1.0 / KVLR)
                    nc.scalar.activation(t1[:, tt:tt + 1], t1[:, tt:tt + 1],
                                         AF.Exp, scale=-0.5)
                    nc.vector.tensor_mul(s_ck[:, tt:tt + 1],
                                         rstd_all[:, tt:tt + 1],
                                         t1[:, tt:tt + 1])
                    nc.vector.tensor_scalar_mul(c_hat[:, tt, 0:512],
                                                ps[:, 0:512],
                                                s_ck[:, tt:tt + 1])
                    nc.vector.tensor_scalar_mul(kr_all[:, tt, :],
                                                ps[:, 512:576],
                                                rstd_all[:, tt:tt + 1])
                # batched k-rope rotation
                krf = p0s.tile([128, NTT, DR], bf16, name="krf")
                x1 = kr_all[:, :, 0:DR:2]
                x2 = kr_all[:, :, 1:DR:2]
                ta = p0s.tile([128, NTT, DR // 2], f32, name="tak")
                tb = p0s.tile([128, NTT, DR // 2], f32, name="tbk")
                nc.vector.tensor_mul(ta[:], x1, cosk_sb[:])
                nc.vector.tensor_mul(tb[:], x2, sink_sb[:])
                nc.vector.tensor_sub(krf[:, :, 0:DR:2], ta[:], tb[:])
                nc.vector.tensor_mul(ta[:], x2, cosk_sb[:])
                nc.vector.tensor_mul(tb[:], x1, sink_sb[:])
                nc.vector.tensor_add(krf[:, :, 1:DR:2], ta[:], tb[:])
                for tt in range(NTT):
                    for rc in range(NKV):
                        tp = p0tp.tile([128, 128], bf16, tag="tp", name="tp")
                        nc.tensor.transpose(
                            tp[:], c_hat[:, tt, rc * 128:(rc + 1) * 128],
                            eye_sb[:])
                        nc.any.tensor_copy(
                            kT_lat[:, rc, tt * 128:(tt + 1) * 128], tp[:])
                    tp = p0tp.tile([128, 128], bf16, tag="tp", name="tp")
                    nc.tensor.transpose(tp[0:64, :], krf[:, tt, :], eye_sb[:])
                    nc.any.tensor_copy(kT_rope[:, tt * 128:(tt + 1) * 128],
                                       tp[0:64, :])

        # =============== phase 2: attention (all in T layouts) ===============
        if MAXPH >= 3:
            with tc.tile_pool(name="p2", bufs=2) as p2, \
                 tc.tile_pool(name="p2d", bufs=2) as p2d, \
                 tc.tile_pool(name="p2e", bufs=3) as p2e, \
                 tc.tile_pool(name="p2ps", bufs=2, space="PSUM") as p2ps, \
                 tc.tile_pool(name="olps", bufs=4, space="PSUM") as olps, \
                 tc.tile_pool(name="denps", bufs=2, space="PSUM") as denps:
                for h in range(NH):
                    qbw = p2.tile([128, NRC * DN], bf16, tag="qbw", name="qbw")
                    nc.sync.dma_start(out=qbw[:], in_=qbn_blk[h])
                    wuk_h = p2.tile([128, NKV, 128], bf16, tag="wuk_h",
                                    name="wuk_h")
                    nc.sync.dma_start(out=wuk_h[:], in_=wuk[h])
                    wuv_h = p2.tile([128, NKV, DV], bf16, tag="wuv_h",
                                    name="wuv_h")
                    nc.sync.dma_start(out=wuv_h[:], in_=wuv[h])

                    # qnT[dn, q] = sum_rc qb_nope[rc].T @ qcT[rc]
                    qn_ps = p2ps.tile([128, RPC], f32, tag="sp", name="qnps")
                    for rc in range(NRC):
                        nc.tensor.matmul(qn_ps[:], qbw[:, rc * DN:(rc + 1) * DN],
                                         qcT[:, rc, :],
                                         start=(rc == 0), stop=(rc == NRC - 1))
                    qnT = p2d.tile([128, RPC], bf16, tag="qnT", name="qnT")
                    nc.scalar.copy(qnT[:], qn_ps[:])
                    # qT_lat[lat, q] = wuk.T @ qnT  (4 latent chunks)
                    qlat = p2d.tile([128, NKV, RPC], bf16, tag="qlat",
                                    name="qlat")
                    for rc in range(NKV):
                        lp = p2ps.tile([128, RPC], f32, tag="sp", name="latps")
                        nc.tensor.matmul(lp[:], wuk_h[:, rc, :], qnT[:],
                                         start=True, stop=True)
                        nc.scalar.copy(qlat[:, rc, :], lp[:])

                    ol = [olps.tile([128, RPC], f32, tag="ol", name="ol")
                          for _ in range(NKV)]
                    den = denps.tile([8, RPC], f32, tag="den", name="den")
                    for kt in range(NTT):
                        w0 = 16 * kt
                        sp = p2ps.tile([128, RPC], f32, tag="sp", name="sp")
                        for dc in range(NKV):
                            nc.tensor.matmul(
                                sp[:, w0:], kT_lat[:, dc, kt * 128:(kt + 1) * 128],
                                qlat[:, dc, w0:], start=(dc == 0), stop=False)
                        nc.tensor.matmul(
                            sp[:, w0:], kT_rope[:, kt * 128:(kt + 1) * 128],
                            qropeT[:, h, w0:], start=False, stop=True)
                        eT = p2e.tile([128, RPC], bf16, tag="eT", name="eT")
                        nc.scalar.activation(eT[:, w0:], sp[:, w0:], AF.Exp)
                        nc.vector.tensor_mul(eT[:, w0:w0 + 16],
                                             eT[:, w0:w0 + 16], mask_sb[:])
                        for rc in range(NKV):
                            nc.tensor.matmul(
                                ol[rc][:, w0:],
                                c_hat[:, kt, rc * 128:(rc + 1) * 128],
                                eT[:, w0:],
                                start=(kt == 0), stop=(kt == NTT - 1))
                        nc.tensor.matmul(
                            den[:, w0:], c_hat[:, kt, 512:CAUG], eT[:, w0:],
                            start=(kt == 0), stop=(kt == NTT - 1))
                    rinv = p2d.tile([1, RPC], bf16, tag="rinv", name="rinv")
                    with nc.allow_low_precision(
                            reason="1/den feeds bf16 matmul rhs anyway"):
                        nc.vector.reciprocal(rinv[:], den[0:1, :])
                    rb_ps = p2ps.tile([128, RPC], f32, tag="sp", name="rbps")
                    nc.tensor.matmul(rb_ps[:], ones_sb[:], rinv[:],
                                     start=True, stop=True)
                    rb = p2d.tile([128, RPC], bf16, tag="rb", name="rb")
                    nc.scalar.copy(rb[:], rb_ps[:])
                    olT = p2d.tile([128, NKV, RPC], bf16, tag="olT", name="olT")
                    for rc in range(NKV):
                        nc.any.tensor_copy(olT[:, rc, :], ol[rc][:])
                    ovp = p2ps.tile([128, RPC], f32, tag="sp", name="ovp")
                    for rc in range(NKV):
                        nc.tensor.matmul(ovp[:], wuv_h[:, rc, :], olT[:, rc, :],
                                         start=(rc == 0), stop=(rc == NKV - 1))
                    nc.vector.tensor_mul(o_vT[:, h, :], ovp[:], rb[:])

        # =============== phase 3: o_proj + residual + post-norm ===============
        if MAXPH >= 4:
            with tc.tile_pool(name="p3", bufs=2) as p3, \
                 tc.tile_pool(name="p3s", bufs=1) as p3s, \
                 tc.tile_pool(name="p3d", bufs=2) as p3d:
                hn = p3s.tile([128, NQT, H], bf16, name="hn")
                with tc.tile_pool(name="p3ps", bufs=2, space="PSUM") as p3ps:
                    op_ps = [p3ps.tile([128, H], f32, tag="opps", name="opps")
                             for _ in range(NQT)]
                    for hc in range(NH):
                        oww = p3.tile([128, H], bf16, tag="oww", name="oww")
                        nc.gpsimd.dma_start(out=oww[:], in_=ow_blk[hc])
                        for qt in range(NQT):
                            for nn in range(4):
                                nc.tensor.matmul(
                                    op_ps[qt][:, nn * 512:(nn + 1) * 512],
                                    o_vT[:, hc, qt * 128:(qt + 1) * 128],
                                    oww[:, nn * 512:(nn + 1) * 512],
                                    start=(hc == 0), stop=(hc == NH - 1))
                    ssq2 = p3s.tile([128, NQT], f32, name="ssq2")
                    for qt in range(NQT):
                        nc.vector.tensor_add(x_rows_sb[:, qt, :],
                                             x_rows_sb[:, qt, :], op_ps[qt][:])
                    for qt in range(NQT):
                        scr = p3d.tile([128, H], bf16, tag="scr3", name="scr3")
                        nc.vector.scalar_tensor_tensor(
                            scr[:], x_rows_sb[:, qt, :], 1.0,
                            x_rows_sb[:, qt, :],
                            ALU.bypass, ALU.mult, accum_out=ssq2[:, qt:qt + 1])
                    nc.scalar.activation(ssq2[:], ssq2[:], AF.Ln,
                                         bias=eps_sb[:], scale=1.0 / H)
                    nc.scalar.activation(ssq2[:], ssq2[:], AF.Exp, scale=-0.5)
                    for qt in range(NQT):
                        nc.vector.tensor_scalar_mul(hn[:, qt, :],
                                                    x_rows_sb[:, qt, :],
                                                    ssq2[:, qt:qt + 1])
                with tc.tile_pool(name="p3tp", bufs=2, space="PSUM") as p3tp:
                    for qt in range(NQT):
                        for fc in range(NFC):
                            tp = p3tp.tile([128, 128], bf16, tag="tp",
                                           name="tp")
                            nc.tensor.transpose(
                                tp[:], hn[:, qt, fc * 128:(fc + 1) * 128],
                                eye_sb[:])
                            nc.any.tensor_copy(
                                hnT[:, fc, qt * 128:(qt + 1) * 128], tp[:])

        # =============== phase 4: MLP ===============
        if MAXPH >= 5:
            with tc.tile_pool(name="p4", bufs=2) as p4, \
                 tc.tile_pool(name="p4ps", bufs=2, space="PSUM") as p4ps:
                GOFF = NFC * 128
                for it in range(NIT):
                    gw = p4.tile([128, 2 * NFC * 128], bf16, tag="gw", name="gw")
                    nc.gpsimd.dma_start(out=gw[:], in_=gu_blk[it])
                    gp = p4ps.tile([128, RPC], f32, tag="gp", name="gp")
                    up = p4ps.tile([128, RPC], f32, tag="up", name="up")
                    for fc in range(NFC):
                        nc.tensor.matmul(gp[:], gw[:, fc * 128:(fc + 1) * 128],
                                         hnT[:, fc, :],
                                         start=(fc == 0), stop=(fc == NFC - 1))
                        nc.tensor.matmul(up[:],
                                         gw[:, GOFF + fc * 128:GOFF + (fc + 1) * 128],
                                         hnT[:, fc, :],
                                         start=(fc == 0), stop=(fc == NFC - 1))
                    gs = p4.tile([128, RPC], bf16, tag="gs", name="gs")
                    nc.scalar.activation(gs[:], gp[:], AF.Silu)
                    nc.vector.tensor_mul(act_all[:, it, :], gs[:], up[:])
            with tc.tile_pool(name="p4b", bufs=2) as p4b, \
                 tc.tile_pool(name="p4s", bufs=2) as p4s, \
                 tc.tile_pool(name="p4bps", bufs=2, space="PSUM") as p4bps:
                o_ps = [p4bps.tile([128, H], f32, tag="ops", name="ops")
                        for _ in range(NQT)]
                for jt in range(NIT // 2):
                    dw = p4b.tile([128, 2 * H], bf16, tag="dw", name="dw")
                    nc.gpsimd.dma_start(out=dw[:], in_=dw_blk[jt])
                    for k in range(2):
                        it = 2 * jt + k
                        for qt in range(NQT):
                            for nn in range(4):
                                nc.tensor.matmul(
                                    o_ps[qt][:, nn * 512:(nn + 1) * 512],
                                    act_all[:, it, qt * 128:(qt + 1) * 128],
                                    dw[:, k * H + nn * 512:k * H + (nn + 1) * 512],
                                    start=(it == 0), stop=(it == NIT - 1))
                for qt in range(NQT):
                    fin = p4s.tile([128, H], f32, tag="fin", name="fin")
                    nc.vector.tensor_add(fin[:], x_rows_sb[:, qt, :],
                                         o_ps[qt][:])
                    nc.sync.dma_start(out=out_rows[qt], in_=fin[:])

        if MAXPH < 5:
            with tc.tile_pool(name="pex", bufs=2) as pex:
                for qt in range(NQT):
                    fin = pex.tile([128, H], f32, tag="finx", name="finx")
                    nc.vector.tensor_copy(fin[:], x_rows_sb[:, qt, :])
                    nc.sync.dma_start(out=out_rows[qt], in_=fin[:])
    nc.compile()
    return nc


def _host_prep(inputs):
    f32 = np.float32
    bf = bfloat16
    x = np.asarray(inputs["hidden_states"], f32)
    pos = np.asarray(inputs["positions"]).astype(f32)

    lnw_in = np.asarray(inputs["input_ln_w"], f32)
    q_a_w = np.asarray(inputs["q_a_w"], f32) * lnw_in[:, None]
    kv_a_w = np.asarray(inputs["kv_a_w"], f32) * lnw_in[:, None]
    q_b_w = (np.asarray(inputs["q_b_w"], f32)
             * np.asarray(inputs["q_a_ln_w"], f32)[:, None]) * SCALE
    kvln = np.asarray(inputs["kv_a_ln_w"], f32)
    w_uk = np.asarray(inputs["w_uk"], f32) * kvln[:, None, None]
    w_uv = np.asarray(inputs["w_uv"], f32) * kvln[:, None, None]
    o_w = np.asarray(inputs["o_w"], f32)
    pln = np.asarray(inputs["post_ln_w"], f32)
    gate_w = np.asarray(inputs["gate_w"], f32) * pln[:, None]
    up_w = np.asarray(inputs["up_w"], f32) * pln[:, None]
    down_w = np.asarray(inputs["down_w"], f32)

    xT = np.ascontiguousarray(x.T)
    inv_freq = 1.0 / (THETA ** (np.arange(0, DR, 2, dtype=f32) / DR))
    ang = pos[:, None] * inv_freq
    cos_t = np.cos(ang).astype(f32)
    sin_t = np.sin(ang).astype(f32)

    gu = np.zeros((2, IPAD, H), f32)
    gu[0, :INTER] = gate_w.T
    gu[1, :INTER] = up_w.T

    qb4 = q_b_w.astype(bf).reshape(NRC, 128, NH, DN + DR)

    rep = {
        "xstat": np.ascontiguousarray(x.reshape(NTT, 128, H).astype(bf)),
        "xT_blk": np.ascontiguousarray(
            xT.astype(bf).reshape(NFC, 128, NTT, 128).transpose(2, 1, 0, 3)),
        "qa_blk": np.ascontiguousarray(q_a_w.astype(bf).reshape(NFC, 128, QLR)),
        # qbn_blk[h, p, rc*DN+d] = SCALE*q_b_w[rc*128+p, h*192+d], d < DN
        "qbn_blk": np.ascontiguousarray(
            qb4[:, :, :, :DN].transpose(2, 1, 0, 3).reshape(NH, 128, NRC * DN)),
        # qbr_blk[rc, p, h*64+j] = SCALE*q_b_w[rc*128+p, h*192+128+j]
        "qbr_blk": np.ascontiguousarray(
            qb4[:, :, :, DN:].reshape(NRC, 128, NH * DR)),
        "kva_blk": np.ascontiguousarray(
            kv_a_w.astype(bf).reshape(NFC, 128, KVLR + DR)),
        "wuk": np.ascontiguousarray(
            w_uk.transpose(1, 2, 0).reshape(NH, 128, NKV, 128).astype(bf)),
        "wuv": np.ascontiguousarray(
            w_uv.transpose(1, 0, 2).reshape(NH, NKV, 128, DV)
            .transpose(0, 2, 1, 3).astype(bf)),
        "ow_blk": np.ascontiguousarray(o_w.astype(bf).reshape(NH, 128, H)),
        # gu_blk[it, hp, g*NFC*128 + fc*128 + ip] = gate/up[fc*128+hp, it*128+ip]
        "gu_blk": np.ascontiguousarray(
            gu.reshape(2, NIT, 128, NFC, 128).transpose(1, 4, 0, 3, 2)
            .reshape(NIT, 128, 2 * NFC * 128).astype(bf)),
        "dw_blk": np.ascontiguousarray(
            np.concatenate([down_w, np.zeros((IPAD - INTER, H), f32)], 0)
            .astype(bf).reshape(NIT // 2, 2, 128, H).transpose(0, 2, 1, 3)
            .reshape(NIT // 2, 128, 2 * H)),
        "cosk": np.ascontiguousarray(
            cos_t.reshape(NTT, 128, DR // 2).transpose(1, 0, 2)),
        "sink": np.ascontiguousarray(
            sin_t.reshape(NTT, 128, DR // 2).transpose(1, 0, 2)),
        "eye": np.eye(128, dtype=bf),
        "ones_row": np.ones((1, 128), bf),
    }

    per_core = []
    kpos = np.arange(128)
    qpos = np.arange(16)
    for c in range(NCORES):
        rows = np.arange(c, T, NCORES)
        m = dict(rep)
        m["x_rows"] = np.ascontiguousarray(x[rows].reshape(NQT, 128, H))
        m["xTc"] = np.ascontiguousarray(
            xT[:, rows].astype(bf).reshape(NFC, 128, RPC))
        cq = np.broadcast_to(
            cos_t[rows].reshape(NQT, 128, 1, DR // 2),
            (NQT, 128, NH, DR // 2))
        sq = np.broadcast_to(
            sin_t[rows].reshape(NQT, 128, 1, DR // 2),
            (NQT, 128, NH, DR // 2))
        m["cosq16"] = np.ascontiguousarray(
            cq.reshape(NQT, 128, NH * DR // 2).astype(bf))
        m["sinq16"] = np.ascontiguousarray(
            sq.reshape(NQT, 128, NH * DR // 2).astype(bf))
        # mask16[j, i] = key j attends-allowed for query row (16*kt + i):
        # key pos 128*kt + j <= query pos 8*(16*kt+i) + c  <=>  j <= 8*i + c
        m["mask16"] = np.ascontiguousarray(
            (kpos[:, None] <= 8 * qpos[None, :] + c).astype(bf))
        per_core.append(m)
    return per_core


def kernel(**inputs):
    from concourse import bass_utils

    if "nc" not in _CACHE:
        _CACHE["nc"] = _build_module()
    nc = _CACHE["nc"]

    import os
    in_maps = _host_prep(inputs)
    trace = bool(os.environ.get("BASS_KERNEL_TRACE"))
    res = bass_utils.run_bass_kernel_spmd(nc, in_maps,
                                          core_ids=list(range(NCORES)),
                                          trace=trace)
    if trace and res.exec_time_ns is not None:
        print(f"HW exec time: {res.exec_time_ns} ns")
        _CACHE["last_result"] = res
    out = np.zeros((T, H), np.float32)
    for c in range(NCORES):
        rows = np.arange(c, T, NCORES)
        out[rows] = res.results[c]["out_rows"].reshape(RPC, H)
    return out


# revision 14
# speedup vs baseline: 1.1143x; 1.1143x over previous
"""DeepseekV3 decoder layer (MLA + SwiGLU MLP), T=2048 prefill, fp32 I/O.

Sharding: sequence-parallel striped - core c owns token rows c::8 (256 rows),
so all 8 cores run one identical SPMD program with balanced causal work.
The KV latent path (all 2048 tokens) is replicated on every core; outputs
are disjoint row sets concatenated on host.

v2 redesign vs v1:
- Causal tightening: with striped rows, key tile kt is attended only by
  query rows >= 16*kt on EVERY core, so score/attn matmuls shrink their
  free dim per key tile (2176 vs 3072 q-cols of work) and masking reduces
  to one [128,16] triangle applied to the first 16 active columns.
- Attention computed fully in transposed layouts: qnT/qT_lat directly from
  qcT (no per-head transposes), o_latent^T accumulated directly via
  lhsT=c_hat tiles, denominator via an appended ones-column in c_hat,
  1/den applied to o_vT via a K=1 broadcast matmul.
- SCALE folded into q_b weights on host; LN weights folded as in v1.
- MLP/o_proj weights streamed on the gpsimd (SWDGE) queue so the big
  weight stream never head-of-line-blocks latency-critical sync-queue
  loads; gate+up fused to 1MB DMAs, down proj 1MB (2 tiles per DMA).
"""

import numpy as np
import ml_dtypes

bfloat16 = ml_dtypes.bfloat16

T = 2048
H = 2048
NH = 16
QLR = 1536
KVLR = 512
DN = 128
DR = 64
DV = 128
INTER = 10944
NCORES = 8
RPC = T // NCORES          # 256 query rows per core
NQT = RPC // 128           # 2
NTT = T // 128             # 16
NFC = H // 128             # 16
NRC = QLR // 128           # 12
NKV = KVLR // 128          # 4
NIT = 86
IPAD = NIT * 128
EPS = 1e-6
SCALE = (DN + DR) ** -0.5
THETA = 10000.0
CAUG = 520                 # c_hat free width: 512 latent + 1 ones + 7 pad

_CACHE = {}


def _build_module():
    import os
    MAXPH = int(os.environ.get("KERNEL_MAXPH", "9"))
    import concourse.bass as bass
    import concourse.tile as tile
    from concourse import bacc, mybir

    f32 = mybir.dt.float32
    bf16 = mybir.dt.bfloat16
    AF = mybir.ActivationFunctionType
    ALU = mybir.AluOpType

    nc = bacc.Bacc("TRN2", target_bir_lowering=False, debug=False,
                   enable_asserts=False, num_devices=NCORES)

    def inp(name, shape, dt):
        return nc.dram_tensor(name, list(shape), dt, kind="ExternalInput").ap()

    # per-core inputs
    x_rows = inp("x_rows", [NQT, 128, H], f32)
    xTc = inp("xTc", [NFC, 128, RPC], bf16)
    cosq16 = inp("cosq16", [NQT, 128, NH * DR // 2], bf16)
    sinq16 = inp("sinq16", [NQT, 128, NH * DR // 2], bf16)
    mask16 = inp("mask16", [128, 16], bf16)
    # replicated inputs
    xstat = inp("xstat", [NTT, 128, H], bf16)
    xT_blk = inp("xT_blk", [NTT, 128, NFC, 128], bf16)
    qa_blk = inp("qa_blk", [NFC, 128, QLR], bf16)
    qbn_blk = inp("qbn_blk", [NH, 128, NRC * DN], bf16)
    qbr_blk = inp("qbr_blk", [NRC, 128, NH * DR], bf16)
    kva_blk = inp("kva_blk", [NFC, 128, KVLR + DR], bf16)
    wuk = inp("wuk", [NH, 128, NKV, 128], bf16)
    wuv = inp("wuv", [NH, 128, NKV, DV], bf16)
    ow_blk = inp("ow_blk", [NH, 128, H], bf16)
    gu_blk = inp("gu_blk", [NIT, 128, 2 * NFC * 128], bf16)
    dw_blk = inp("dw_blk", [NIT // 2, 128, 2 * H], bf16)
    cosk = inp("cosk", [128, NTT, DR // 2], f32)
    sink = inp("sink", [128, NTT, DR // 2], f32)
    eye = inp("eye", [128, 128], bf16)
    ones_row = inp("ones_row", [1, 128], bf16)

    out_rows = nc.dram_tensor("out_rows", [NQT, 128, H], f32,
                              kind="ExternalOutput").ap()

    from contextlib import ExitStack
    with tile.TileContext(nc) as tc, ExitStack() as ctx:
        persist = ctx.enter_context(tc.tile_pool(name="persist", bufs=1))

        def pt(shape, dt, tag):
            return persist.tile(list(shape), dt, tag=tag, name=tag)

        eps_sb = pt([128, 1], f32, "eps")
        nc.vector.memset(eps_sb[:], EPS)
        eye_sb = pt([128, 128], bf16, "eye")
        nc.sync.dma_start(out=eye_sb[:], in_=eye[:])
        ones_sb = pt([1, 128], bf16, "ones")
        nc.sync.dma_start(out=ones_sb[:], in_=ones_row[:])
        mask_sb = pt([128, 16], bf16, "mask")
        nc.sync.dma_start(out=mask_sb[:], in_=mask16[:])
        x_rows_sb = pt([128, NQT, H], f32, "x_rows")
        for qt in range(NQT):
            nc.sync.dma_start(out=x_rows_sb[:, qt, :], in_=x_rows[qt])
        cq_sb = pt([128, NQT, NH * DR // 2], bf16, "cq")
        sq_sb = pt([128, NQT, NH * DR // 2], bf16, "sq")
        for qt in range(NQT):
            nc.sync.dma_start(out=cq_sb[:, qt, :], in_=cosq16[qt])
            nc.sync.dma_start(out=sq_sb[:, qt, :], in_=sinq16[qt])

        rstd_all = pt([128, NTT], f32, "rstd_all")
        c_hat = pt([128, NTT, CAUG], bf16, "c_hat")
        # ones column + zero pad for the denominator trick
        nc.vector.memset(c_hat[:, :, 512:513], 1.0)
        nc.vector.memset(c_hat[:, :, 513:CAUG], 0.0)
        kT_lat = pt([128, NKV, T], bf16, "kT_lat")
        kT_rope = pt([64, T], bf16, "kT_rope")
        qcT = pt([128, NRC, RPC], bf16, "qcT")
        qropeT = pt([64, NH, RPC], bf16, "qropeT")
        o_vT = pt([128, NH, RPC], bf16, "o_vT")
        hnT = pt([128, NFC, RPC], bf16, "hnT")
        act_all = pt([128, NIT, RPC], bf16, "act_all")

        # =============== phase B1: row stats + q_a GEMM ===============
        with tc.tile_pool(name="pb", bufs=2) as pb, \
             tc.tile_pool(name="pbs", bufs=1) as pbs, \
             tc.tile_pool(name="pbd", bufs=2) as pbd, \
             tc.tile_pool(name="pbps", bufs=2, space="PSUM") as pbps, \
             tc.tile_pool(name="pbtp", bufs=2, space="PSUM") as pbtp:
            rstd_rows = pbs.tile([128, NQT], f32, name="rstd_rows")
            ssq_r = pbs.tile([128, NQT], f32, name="ssq_r")
            for qt in range(NQT):
                scrap = pbd.tile([128, H], bf16, tag="scrapq", name="scrapq")
                nc.vector.scalar_tensor_tensor(
                    scrap[:], x_rows_sb[:, qt, :], 1.0, x_rows_sb[:, qt, :],
                    ALU.bypass, ALU.mult, accum_out=ssq_r[:, qt:qt + 1])
            nc.scalar.activation(rstd_rows[:], ssq_r[:], AF.Ln,
                                 bias=eps_sb[:], scale=1.0 / H)
            nc.scalar.activation(rstd_rows[:], rstd_rows[:], AF.Exp, scale=-0.5)

            xTc_sb = pbs.tile([128, NFC, RPC], bf16, name="xTc_sb")
            for fc in range(NFC):
                nc.sync.dma_start(out=xTc_sb[:, fc, :], in_=xTc[fc])
            qa_ps = [pbps.tile([128, QLR], f32, tag="qaps", name="qaps")
                     for _ in range(NQT)]
            for fc in range(NFC):
                qaw = pb.tile([128, QLR], bf16, tag="qaw", name="qaw")
                nc.sync.dma_start(out=qaw[:], in_=qa_blk[fc])
                for qt in range(NQT):
                    for nn in range(QLR // 512):
                        nc.tensor.matmul(
                            qa_ps[qt][:, nn * 512:(nn + 1) * 512],
                            xTc_sb[:, fc, qt * 128:(qt + 1) * 128],
                            qaw[:, nn * 512:(nn + 1) * 512],
                            start=(fc == 0), stop=(fc == NFC - 1))
            qc = pbs.tile([128, NQT, QLR], bf16, name="qc")
            ssq_q = pbs.tile([128, NQT], f32, name="ssq_q")
            for qt in range(NQT):
                scr = pbd.tile([128, QLR], bf16, tag="scrq2", name="scrq2")
                nc.scalar.activation(scr[:], qa_ps[qt][:], AF.Square,
                                     accum_out=ssq_q[:, qt:qt + 1])
            sq = pbs.tile([128, NQT], f32, name="sqn")
            nc.vector.tensor_mul(sq[:], rstd_rows[:], rstd_rows[:])
            nc.vector.tensor_mul(sq[:], sq[:], ssq_q[:])
            nc.scalar.activation(sq[:], sq[:], AF.Ln, bias=eps_sb[:],
                                 scale=1.0 / QLR)
            nc.scalar.activation(sq[:], sq[:], AF.Exp, scale=-0.5)
            nc.vector.tensor_mul(sq[:], rstd_rows[:], sq[:])
            for qt in range(NQT):
                nc.vector.tensor_scalar_mul(qc[:, qt, :], qa_ps[qt][:],
                                            sq[:, qt:qt + 1])
            for qt in range(NQT):
                for rc in range(NRC):
                    tp = pbtp.tile([128, 128], bf16, tag="tp", name="tp")
                    nc.tensor.transpose(tp[:], qc[:, qt, rc * 128:(rc + 1) * 128],
                                        eye_sb[:])
                    nc.any.tensor_copy(qcT[:, rc, qt * 128:(qt + 1) * 128],
                                       tp[:])

        # =============== phase B2: rope-q for all heads ===============
        if MAXPH >= 1:
            with tc.tile_pool(name="pr", bufs=2) as pr, \
                 tc.tile_pool(name="prs", bufs=1) as prs, \
                 tc.tile_pool(name="prd", bufs=2) as prd, \
                 tc.tile_pool(name="prps", bufs=2, space="PSUM") as prps, \
                 tc.tile_pool(name="prtp", bufs=2, space="PSUM") as prtp:
                qbr_sb = prs.tile([128, NRC, NH * DR], bf16, name="qbr_sb")
                for rc in range(NRC):
                    nc.sync.dma_start(out=qbr_sb[:, rc, :], in_=qbr_blk[rc])
                qr_sb = prs.tile([128, NQT, NH * DR], bf16, name="qr_sb")
                ND = NH * DR
                for qt in range(NQT):
                    rps = prps.tile([128, ND], f32, tag="rps", name="rps")
                    for rc in range(NRC):
                        for half in range(2):
                            nc.tensor.matmul(
                                rps[:, half * 512:(half + 1) * 512],
                                qcT[:, rc, qt * 128:(qt + 1) * 128],
                                qbr_sb[:, rc, half * 512:(half + 1) * 512],
                                start=(rc == 0), stop=(rc == NRC - 1))
                    # rotate pairs: flat col h*64+2i <-> (head h, pair i);
                    # cq/sq are packed h-major [128, NH*32] to match
                    x1 = rps[:, 0:ND:2]
                    x2 = rps[:, 1:ND:2]
                    cs = cq_sb[:, qt, :]
                    sn = sq_sb[:, qt, :]
                    ta = prd.tile([128, ND // 2], f32, tag="ta", name="ta")
                    tb = prd.tile([128, ND // 2], f32, tag="tb", name="tb")
                    nc.vector.tensor_mul(ta[:], x1, cs)
                    nc.vector.tensor_mul(tb[:], x2, sn)
                    nc.vector.tensor_sub(qr_sb[:, qt, 0:ND:2], ta[:], tb[:])
                    nc.vector.tensor_mul(ta[:], x2, cs)
                    nc.vector.tensor_mul(tb[:], x1, sn)
                    nc.vector.tensor_add(qr_sb[:, qt, 1:ND:2], ta[:], tb[:])
                for qt in range(NQT):
                    for h in range(NH):
                        tp = prtp.tile([128, 128], bf16, tag="tp", name="tp")
                        nc.tensor.transpose(
                            tp[0:64, :],
                            qr_sb[:, qt, h * DR:(h + 1) * DR], eye_sb[:])
                        nc.any.tensor_copy(
                            qropeT[:, h, qt * 128:(qt + 1) * 128], tp[0:64, :])

        # =============== phase 0: all-token stats + kv path ===============
        if MAXPH >= 2:
            with tc.tile_pool(name="p0", bufs=2) as p0, \
                 tc.tile_pool(name="p0x", bufs=2) as p0x, \
                 tc.tile_pool(name="p0w", bufs=1) as p0w, \
                 tc.tile_pool(name="p0s", bufs=1) as p0s, \
                 tc.tile_pool(name="p0d", bufs=2) as p0d, \
                 tc.tile_pool(name="p0ps", bufs=2, space="PSUM") as p0ps, \
                 tc.tile_pool(name="p0tp", bufs=2, space="PSUM") as p0tp:
                cosk_sb = p0s.tile([128, NTT, DR // 2], f32, name="cosk_sb")
                nc.sync.dma_start(out=cosk_sb[:], in_=cosk[:])
                sink_sb = p0s.tile([128, NTT, DR // 2], f32, name="sink_sb")
                nc.sync.dma_start(out=sink_sb[:], in_=sink[:])
                ssq_all = p0s.tile([128, NTT], f32, name="ssq_all")
                for tt in range(NTT):
                    xs = p0x.tile([128, H], bf16, tag="xs", name="xs")
                    nc.sync.dma_start(out=xs[:], in_=xstat[tt])
                    scrap = p0d.tile([128, H], bf16, tag="scrap", name="scrap")
                    nc.vector.scalar_tensor_tensor(
                        scrap[:], xs[:], 1.0, xs[:], ALU.bypass, ALU.mult,
                        accum_out=ssq_all[:, tt:tt + 1])
                nc.scalar.activation(rstd_all[:], ssq_all[:], AF.Ln,
                                     bias=eps_sb[:], scale=1.0 / H)
                nc.scalar.activation(rstd_all[:], rstd_all[:], AF.Exp,
                                     scale=-0.5)

                kvw = p0w.tile([128, NFC, KVLR + DR], bf16, name="kvw")
                for fc in range(NFC):
                    nc.sync.dma_start(out=kvw[:, fc, :], in_=kva_blk[fc])
                kr_all = p0s.tile([128, NTT, DR], bf16, name="kr_all")
                ssq_kv = p0s.tile([128, NTT], f32, name="ssq_kv")
                s_ck = p0s.tile([128, NTT], f32, name="s_ck")
                t1 = p0s.tile([128, NTT], f32, name="t1")
                # stage RAW kv outputs (c_hat holds unscaled latent for now;
                # scales applied in one batch below so the GEMM never stalls
                # on the stats chain and Ln/Exp tables load once)
                for tt in range(NTT):
                    xt = p0.tile([128, NFC, 128], bf16, tag="xt", name="xt")
                    nc.sync.dma_start(out=xt[:], in_=xT_blk[tt])
                    ps = p0ps.tile([128, KVLR + DR], f32, tag="kvps",
                                   name="kvps")
                    for fc in range(NFC):
                        nc.tensor.matmul(ps[:, 0:512], xt[:, fc, :],
                                         kvw[:, fc, 0:512],
                                         start=(fc == 0), stop=(fc == NFC - 1))
                        nc.tensor.matmul(ps[:, 512:576], xt[:, fc, :],
                                         kvw[:, fc, 512:576],
                                         start=(fc == 0), stop=(fc == NFC - 1))
                    scr2 = p0d.tile([128, KVLR], bf16, tag="scr2", name="scr2")
                    nc.scalar.activation(scr2[:], ps[:, 0:512], AF.Square,
                                         accum_out=ssq_kv[:, tt:tt + 1])
                    nc.vector.tensor_copy(c_hat[:, tt, 0:512], ps[:, 0:512])
                    nc.vector.tensor_copy(kr_all[:, tt, :], ps[:, 512:576])
                # batch: s_ck = rstd * rsqrt(mean((rstd*kv)^2)+eps)
                nc.vector.tensor_mul(t1[:], rstd_all[:], rstd_all[:])
                nc.vector.tensor_mul(t1[:], t1[:], ssq_kv[:])
                nc.scalar.activation(t1[:], t1[:], AF.Ln, bias=eps_sb[:],
                                     scale=1.0 / KVLR)
                nc.scalar.activation(t1[:], t1[:], AF.Exp, scale=-0.5)
                nc.vector.tensor_mul(s_ck[:], rstd_all[:], t1[:])
                for tt in range(NTT):
                    nc.vector.tensor_scalar_mul(c_hat[:, tt, 0:512],
                                                c_hat[:, tt, 0:512],
                                                s_ck[:, tt:tt + 1])
                    nc.vector.tensor_scalar_mul(kr_all[:, tt, :],
                                                kr_all[:, tt, :],
                                                rstd_all[:, tt:tt + 1])
                # batched k-rope rotation
                krf = p0s.tile([128, NTT, DR], bf16, name="krf")
                x1 = kr_all[:, :, 0:DR:2]
                x2 = kr_all[:, :, 1:DR:2]
                ta = p0s.tile([128, NTT, DR // 2], f32, name="tak")
                tb = p0s.tile([128, NTT, DR // 2], f32, name="tbk")
                nc.vector.tensor_mul(ta[:], x1, cosk_sb[:])
                nc.vector.tensor_mul(tb[:], x2, sink_sb[:])
                nc.vector.tensor_sub(krf[:, :, 0:DR:2], ta[:], tb[:])
                nc.vector.tensor_mul(ta[:], x2, cosk_sb[:])
                nc.vector.tensor_mul(tb[:], x1, sink_sb[:])
                nc.vector.tensor_add(krf[:, :, 1:DR:2], ta[:], tb[:])
                for tt in range(NTT):
                    for rc in range(NKV):
                        tp = p0tp.tile([128, 128], bf16, tag="tp", name="tp")
                        nc.tensor.transpose(
                            tp[:], c_hat[:, tt, rc * 128:(rc + 1) * 128],
                            eye_sb[:])
                        nc.any.tensor_copy(
                            kT_lat[:, rc, tt * 128:(tt + 1) * 128], tp[:])
                    tp = p0tp.tile([128, 128], bf16, tag="tp", name="tp")
                    nc.tensor.transpose(tp[0:64, :], krf[:, tt, :], eye_sb[:])
                    nc.any.tensor_copy(kT_rope[:, tt * 128:(tt + 1) * 128],
                                       tp[0:64, :])

        # =============== phase 2: attention (all in T layouts) ===============
        if MAXPH >= 3:
            with tc.tile_pool(name="p2", bufs=2) as p2, \
                 tc.tile_pool(name="p2d", bufs=2) as p2d, \
                 tc.tile_pool(name="p2e", bufs=3) as p2e, \
                 tc.tile_pool(name="p2ps", bufs=2, space="PSUM") as p2ps, \
                 tc.tile_pool(name="olps", bufs=4, space="PSUM") as olps, \
                 tc.tile_pool(name="denps", bufs=2, space="PSUM") as denps:
                for h in range(NH):
                    qbw = p2.tile([128, NRC * DN], bf16, tag="qbw", name="qbw")
                    nc.gpsimd.dma_start(out=qbw[:], in_=qbn_blk[h])
                    wuk_h = p2.tile([128, NKV, 128], bf16, tag="wuk_h",
                                    name="wuk_h")
                    nc.gpsimd.dma_start(out=wuk_h[:], in_=wuk[h])
                    wuv_h = p2.tile([128, NKV, DV], bf16, tag="wuv_h",
                                    name="wuv_h")
                    nc.gpsimd.dma_start(out=wuv_h[:], in_=wuv[h])

                    # qnT[dn, q] = sum_rc qb_nope[rc].T @ qcT[rc]
                    qn_ps = p2ps.tile([128, RPC], f32, tag="sp", name="qnps")
                    for rc in range(NRC):
                        nc.tensor.matmul(qn_ps[:], qbw[:, rc * DN:(rc + 1) * DN],
                                         qcT[:, rc, :],
                                         start=(rc == 0), stop=(rc == NRC - 1))
                    qnT = p2d.tile([128, RPC], bf16, tag="qnT", name="qnT")
                    nc.scalar.copy(qnT[:], qn_ps[:])
                    # qT_lat[lat, q] = wuk.T @ qnT  (4 latent chunks)
                    qlat = p2d.tile([128, NKV, RPC], bf16, tag="qlat",
                                    name="qlat")
                    for rc in range(NKV):
                        lp = p2ps.tile([128, RPC], f32, tag="sp", name="latps")
                        nc.tensor.matmul(lp[:], wuk_h[:, rc, :], qnT[:],
                                         start=True, stop=True)
                        nc.scalar.copy(qlat[:, rc, :], lp[:])

                    ol = [olps.tile([128, RPC], f32, tag="ol", name="ol")
                          for _ in range(NKV)]
                    den = denps.tile([8, RPC], f32, tag="den", name="den")
                    for kt in range(NTT):
                        w0 = 16 * kt
                        sp = p2ps.tile([128, RPC], f32, tag="sp", name="sp")
                        for dc in range(NKV):
                            nc.tensor.matmul(
                                sp[:, w0:], kT_lat[:, dc, kt * 128:(kt + 1) * 128],
                                qlat[:, dc, w0:], start=(dc == 0), stop=False)
                        nc.tensor.matmul(
                            sp[:, w0:], kT_rope[:, kt * 128:(kt + 1) * 128],
                            qropeT[:, h, w0:], start=False, stop=True)
                        eT = p2e.tile([128, RPC], bf16, tag="eT", name="eT")
                        nc.scalar.activation(eT[:, w0:], sp[:, w0:], AF.Exp)
                        nc.vector.tensor_mul(eT[:, w0:w0 + 16],
                                             eT[:, w0:w0 + 16], mask_sb[:])
                        for rc in range(NKV):
                            nc.tensor.matmul(
                                ol[rc][:, w0:],
                                c_hat[:, kt, rc * 128:(rc + 1) * 128],
                                eT[:, w0:],
                                start=(kt == 0), stop=(kt == NTT - 1))
                        nc.tensor.matmul(
                            den[:, w0:], c_hat[:, kt, 512:CAUG], eT[:, w0:],
                            start=(kt == 0), stop=(kt == NTT - 1))
                    rinv = p2d.tile([1, RPC], bf16, tag="rinv", name="rinv")
                    with nc.allow_low_precision(
                            reason="1/den feeds bf16 matmul rhs anyway"):
                        nc.vector.reciprocal(rinv[:], den[0:1, :])
                    rb_ps = p2ps.tile([128, RPC], f32, tag="sp", name="rbps")
                    nc.tensor.matmul(rb_ps[:], ones_sb[:], rinv[:],
                                     start=True, stop=True)
                    rb = p2d.tile([128, RPC], bf16, tag="rb", name="rb")
                    nc.scalar.copy(rb[:], rb_ps[:])
                    olT = p2d.tile([128, NKV, RPC], bf16, tag="olT", name="olT")
                    for rc in range(NKV):
                        nc.any.tensor_copy(olT[:, rc, :], ol[rc][:])
                    ovp = p2ps.tile([128, RPC], f32, tag="sp", name="ovp")
                    for rc in range(NKV):
                        nc.tensor.matmul(ovp[:], wuv_h[:, rc, :], olT[:, rc, :],
                                         start=(rc == 0), stop=(rc == NKV - 1))
                    nc.vector.tensor_mul(o_vT[:, h, :], ovp[:], rb[:])

        # =============== phase 3: o_proj + residual + post-norm ===============
        if MAXPH >= 4:
            with tc.tile_pool(name="p3", bufs=2) as p3, \
                 tc.tile_pool(name="p3s", bufs=1) as p3s, \
                 tc.tile_pool(name="p3d", bufs=2) as p3d:
                hn = p3s.tile([128, NQT, H], bf16, name="hn")
                with tc.tile_pool(name="p3ps", bufs=2, space="PSUM") as p3ps:
                    op_ps = [p3ps.tile([128, H], f32, tag="opps", name="opps")
                             for _ in range(NQT)]
                    for hc in range(NH):
                        oww = p3.tile([128, H], bf16, tag="oww", name="oww")
                        nc.sync.dma_start(out=oww[:], in_=ow_blk[hc])
                        for qt in range(NQT):
                            for nn in range(4):
                                nc.tensor.matmul(
                                    op_ps[qt][:, nn * 512:(nn + 1) * 512],
                                    o_vT[:, hc, qt * 128:(qt + 1) * 128],
                                    oww[:, nn * 512:(nn + 1) * 512],
                                    start=(hc == 0), stop=(hc == NH - 1))
                    ssq2 = p3s.tile([128, NQT], f32, name="ssq2")
                    for qt in range(NQT):
                        nc.vector.tensor_add(x_rows_sb[:, qt, :],
                                             x_rows_sb[:, qt, :], op_ps[qt][:])
                    for qt in range(NQT):
                        scr = p3d.tile([128, H], bf16, tag="scr3", name="scr3")
                        nc.vector.scalar_tensor_tensor(
                            scr[:], x_rows_sb[:, qt, :], 1.0,
                            x_rows_sb[:, qt, :],
                            ALU.bypass, ALU.mult, accum_out=ssq2[:, qt:qt + 1])
                    nc.scalar.activation(ssq2[:], ssq2[:], AF.Ln,
                                         bias=eps_sb[:], scale=1.0 / H)
                    nc.scalar.activation(ssq2[:], ssq2[:], AF.Exp, scale=-0.5)
                    for qt in range(NQT):
                        nc.vector.tensor_scalar_mul(hn[:, qt, :],
                                                    x_rows_sb[:, qt, :],
                                                    ssq2[:, qt:qt + 1])
                with tc.tile_pool(name="p3tp", bufs=2, space="PSUM") as p3tp:
                    for qt in range(NQT):
                        for fc in range(NFC):
                            tp = p3tp.tile([128, 128], bf16, tag="tp",
                                           name="tp")
                            nc.tensor.transpose(
                                tp[:], hn[:, qt, fc * 128:(fc + 1) * 128],
                                eye_sb[:])
                            nc.any.tensor_copy(
                                hnT[:, fc, qt * 128:(qt + 1) * 128], tp[:])

        # =============== phase 4: MLP ===============
        if MAXPH >= 5:
            with tc.tile_pool(name="p4", bufs=2) as p4, \
                 tc.tile_pool(name="p4ps", bufs=2, space="PSUM") as p4ps:
                GOFF = NFC * 128
                for it in range(NIT):
                    gw = p4.tile([128, 2 * NFC * 128], bf16, tag="gw", name="gw")
                    nc.sync.dma_start(out=gw[:], in_=gu_blk[it])
                    gp = p4ps.tile([128, RPC], f32, tag="gp", name="gp")
                    up = p4ps.tile([128, RPC], f32, tag="up", name="up")
                    for fc in range(NFC):
                        nc.tensor.matmul(gp[:], gw[:, fc * 128:(fc + 1) * 128],
                                         hnT[:, fc, :],
                                         start=(fc == 0), stop=(fc == NFC - 1))
                        nc.tensor.matmul(up[:],
                                         gw[:, GOFF + fc * 128:GOFF + (fc + 1) * 128],
                                         hnT[:, fc, :],
                                         start=(fc == 0), stop=(fc == NFC - 1))
                    gs = p4.tile([128, RPC], bf16, tag="gs", name="gs")
                    nc.scalar.activation(gs[:], gp[:], AF.Silu)
                    nc.vector.tensor_mul(act_all[:, it, :], gs[:], up[:])
            with tc.tile_pool(name="p4b", bufs=2) as p4b, \
                 tc.tile_pool(name="p4s", bufs=2) as p4s, \
                 tc.tile_pool(name="p4bps", bufs=2, space="PSUM") as p4bps:
                o_ps = [p4bps.tile([128, H], f32, tag="ops", name="ops")
                        for _ in range(NQT)]
                for jt in range(NIT // 2):
                    dw = p4b.tile([128, 2 * H], bf16, tag="dw", name="dw")
                    nc.sync.dma_start(out=dw[:], in_=dw_blk[jt])
                    for k in range(2):
                        it = 2 * jt + k
                        for qt in range(NQT):
                            for nn in range(4):
                                nc.tensor.matmul(
                                    o_ps[qt][:, nn * 512:(nn + 1) * 512],
                                    act_all[:, it, qt * 128:(qt + 1) * 128],
                                    dw[:, k * H + nn * 512:k * H + (nn + 1) * 512],
                                    start=(it == 0), stop=(it == NIT - 1))
                for qt in range(NQT):
                    fin = p4s.tile([128, H], f32, tag="fin", name="fin")
                    nc.vector.tensor_add(fin[:], x_rows_sb[:, qt, :],
                                         o_ps[qt][:])
                    nc.sync.dma_start(out=out_rows[qt], in_=fin[:])

        if MAXPH < 5:
            with tc.tile_pool(name="pex", bufs=2) as pex:
                for qt in range(NQT):
                    fin = pex.tile([128, H], f32, tag="finx", name="finx")
                    nc.vector.tensor_copy(fin[:], x_rows_sb[:, qt, :])
                    nc.sync.dma_start(out=out_rows[qt], in_=fin[:])
    nc.compile()
    return nc


def _host_prep(inputs):
    f32 = np.float32
    bf = bfloat16
    x = np.asarray(inputs["hidden_states"], f32)
    pos = np.asarray(inputs["positions"]).astype(f32)

    lnw_in = np.asarray(inputs["input_ln_w"], f32)
    q_a_w = np.asarray(inputs["q_a_w"], f32) * lnw_in[:, None]
    kv_a_w = np.asarray(inputs["kv_a_w"], f32) * lnw_in[:, None]
    q_b_w = (np.asarray(inputs["q_b_w"], f32)
             * np.asarray(inputs["q_a_ln_w"], f32)[:, None]) * SCALE
    kvln = np.asarray(inputs["kv_a_ln_w"], f32)
    w_uk = np.asarray(inputs["w_uk"], f32) * kvln[:, None, None]
    w_uv = np.asarray(inputs["w_uv"], f32) * kvln[:, None, None]
    o_w = np.asarray(inputs["o_w"], f32)
    pln = np.asarray(inputs["post_ln_w"], f32)
    gate_w = np.asarray(inputs["gate_w"], f32) * pln[:, None]
    up_w = np.asarray(inputs["up_w"], f32) * pln[:, None]
    down_w = np.asarray(inputs["down_w"], f32)

    xT = np.ascontiguousarray(x.T)
    inv_freq = 1.0 / (THETA ** (np.arange(0, DR, 2, dtype=f32) / DR))
    ang = pos[:, None] * inv_freq
    cos_t = np.cos(ang).astype(f32)
    sin_t = np.sin(ang).astype(f32)

    gu = np.zeros((2, IPAD, H), f32)
    gu[0, :INTER] = gate_w.T
    gu[1, :INTER] = up_w.T

    qb4 = q_b_w.astype(bf).reshape(NRC, 128, NH, DN + DR)

    rep = {
        "xstat": np.ascontiguousarray(x.reshape(NTT, 128, H).astype(bf)),
        "xT_blk": np.ascontiguousarray(
            xT.astype(bf).reshape(NFC, 128, NTT, 128).transpose(2, 1, 0, 3)),
        "qa_blk": np.ascontiguousarray(q_a_w.astype(bf).reshape(NFC, 128, QLR)),
        # qbn_blk[h, p, rc*DN+d] = SCALE*q_b_w[rc*128+p, h*192+d], d < DN
        "qbn_blk": np.ascontiguousarray(
            qb4[:, :, :, :DN].transpose(2, 1, 0, 3).reshape(NH, 128, NRC * DN)),
        # qbr_blk[rc, p, h*64+j] = SCALE*q_b_w[rc*128+p, h*192+128+j]
        "qbr_blk": np.ascontiguousarray(
            qb4[:, :, :, DN:].reshape(NRC, 128, NH * DR)),
        "kva_blk": np.ascontiguousarray(
            kv_a_w.astype(bf).reshape(NFC, 128, KVLR + DR)),
        "wuk": np.ascontiguousarray(
            w_uk.transpose(1, 2, 0).reshape(NH, 128, NKV, 128).astype(bf)),
        "wuv": np.ascontiguousarray(
            w_uv.transpose(1, 0, 2).reshape(NH, NKV, 128, DV)
            .transpose(0, 2, 1, 3).astype(bf)),
        "ow_blk": np.ascontiguousarray(o_w.astype(bf).reshape(NH, 128, H)),
        # gu_blk[it, hp, g*NFC*128 + fc*128 + ip] = gate/up[fc*128+hp, it*128+ip]
        "gu_blk": np.ascontiguousarray(
            gu.reshape(2, NIT, 128, NFC, 128).transpose(1, 4, 0, 3, 2)
            .reshape(NIT, 128, 2 * NFC * 128).astype(bf)),
        "dw_blk": np.ascontiguousarray(
            np.concatenate([down_w, np.zeros((IPAD - INTER, H), f32)], 0)
            .astype(bf).reshape(NIT // 2, 2, 128, H).transpose(0, 2, 1, 3)
            .reshape(NIT // 2, 128, 2 * H)),
        "cosk": np.ascontiguousarray(
            cos_t.reshape(NTT, 128, DR // 2).transpose(1, 0, 2)),
        "sink": np.ascontiguousarray(
            sin_t.reshape(NTT, 128, DR // 2).transpose(1, 0, 2)),
        "eye": np.eye(128, dtype=bf),
        "ones_row": np.ones((1, 128), bf),
    }

    per_core = []
    kpos = np.arange(128)
    qpos = np.arange(16)
    for c in range(NCORES):
        rows = np.arange(c, T, NCORES)
        m = dict(rep)
        m["x_rows"] = np.ascontiguousarray(x[rows].reshape(NQT, 128, H))
        m["xTc"] = np.ascontiguousarray(
            xT[:, rows].astype(bf).reshape(NFC, 128, RPC))
        cq = np.broadcast_to(
            cos_t[rows].reshape(NQT, 128, 1, DR // 2),
            (NQT, 128, NH, DR // 2))
        sq = np.broadcast_to(
            sin_t[rows].reshape(NQT, 128, 1, DR // 2),
            (NQT, 128, NH, DR // 2))
        m["cosq16"] = np.ascontiguousarray(
            cq.reshape(NQT, 128, NH * DR // 2).astype(bf))
        m["sinq16"] = np.ascontiguousarray(
            sq.reshape(NQT, 128, NH * DR // 2).astype(bf))
        # mask16[j, i] = key j attends-allowed for query row (16*kt + i):
        # key pos 128*kt + j <= query pos 8*(16*kt+i) + c  <=>  j <= 8*i + c
        m["mask16"] = np.ascontiguousarray(
            (kpos[:, None] <= 8 * qpos[None, :] + c).astype(bf))
        per_core.append(m)
    return per_core


def kernel(**inputs):
    from concourse import bass_utils

    if "nc" not in _CACHE:
        _CACHE["nc"] = _build_module()
    nc = _CACHE["nc"]

    import os
    in_maps = _host_prep(inputs)
    trace = bool(os.environ.get("BASS_KERNEL_TRACE"))
    res = bass_utils.run_bass_kernel_spmd(nc, in_maps,
                                          core_ids=list(range(NCORES)),
                                          trace=trace)
    if trace and res.exec_time_ns is not None:
        print(f"HW exec time: {res.exec_time_ns} ns")
        _CACHE["last_result"] = res
    out = np.zeros((T, H), np.float32)
    for c in range(NCORES):
        rows = np.arange(c, T, NCORES)
        out[rows] = res.results[c]["out_rows"].reshape(RPC, H)
    return out


# revision 18
# speedup vs baseline: 1.3520x; 1.2134x over previous
"""DeepseekV3 decoder layer (MLA + SwiGLU MLP), T=2048 prefill, fp32 I/O.

Sharding: sequence-parallel striped - core c owns token rows c::8 (256 rows),
so all 8 cores run one identical SPMD program with balanced causal work.
The KV latent path (all 2048 tokens) is replicated on every core; outputs
are disjoint row sets concatenated on host.

v2 redesign vs v1:
- Causal tightening: with striped rows, key tile kt is attended only by
  query rows >= 16*kt on EVERY core, so score/attn matmuls shrink their
  free dim per key tile (2176 vs 3072 q-cols of work) and masking reduces
  to one [128,16] triangle applied to the first 16 active columns.
- Attention computed fully in transposed layouts: qnT/qT_lat directly from
  qcT (no per-head transposes), o_latent^T accumulated directly via
  lhsT=c_hat tiles, denominator via an appended ones-column in c_hat,
  1/den applied to o_vT via a K=1 broadcast matmul.
- SCALE folded into q_b weights on host; LN weights folded as in v1.
- MLP/o_proj weights streamed on the gpsimd (SWDGE) queue so the big
  weight stream never head-of-line-blocks latency-critical sync-queue
  loads; gate+up fused to 1MB DMAs, down proj 1MB (2 tiles per DMA).
"""

import numpy as np
import ml_dtypes

bfloat16 = ml_dtypes.bfloat16

T = 2048
H = 2048
NH = 16
QLR = 1536
KVLR = 512
DN = 128
DR = 64
DV = 128
INTER = 10944
NCORES = 8
RPC = T // NCORES          # 256 query rows per core
NQT = RPC // 128           # 2
NTT = T // 128             # 16
NFC = H // 128             # 16
NRC = QLR // 128           # 12
NKV = KVLR // 128          # 4
NIT = 86
IPAD = NIT * 128
EPS = 1e-6
SCALE = (DN + DR) ** -0.5
THETA = 10000.0
CAUG = 520                 # c_hat free width: 512 latent + 1 ones + 7 pad

_CACHE = {}


def _build_module():
    import os
    MAXPH = int(os.environ.get("KERNEL_MAXPH", "9"))
    import concourse.bass as bass
    import concourse.tile as tile
    from concourse import bacc, mybir

    f32 = mybir.dt.float32
    bf16 = mybir.dt.bfloat16
    AF = mybir.ActivationFunctionType
    ALU = mybir.AluOpType

    nc = bacc.Bacc("TRN2", target_bir_lowering=False, debug=False,
                   enable_asserts=False, num_devices=NCORES)

    def inp(name, shape, dt):
        return nc.dram_tensor(name, list(shape), dt, kind="ExternalInput").ap()

    # per-core inputs
    x_rows = inp("x_rows", [NQT, 128, H], f32)
    xTc = inp("xTc", [128, NFC * RPC], bf16)
    cosq16 = inp("cosq16", [NQT, 128, NH * DR // 2], bf16)
    sinq16 = inp("sinq16", [NQT, 128, NH * DR // 2], bf16)
    mask16 = inp("mask16", [128, 16], bf16)
    # replicated inputs
    xT_blk = inp("xT_blk", [NTT // 2, 128, 2, NFC, 128], bf16)
    qa_blk = inp("qa_blk", [NFC // 4, 128, 4 * QLR], bf16)
    qbn_blk = inp("qbn_blk", [NH, 128, NRC * DN], bf16)
    qbr_blk = inp("qbr_blk", [NRC // 3, 128, 3 * NH * DR], bf16)
    kva_blk = inp("kva_blk", [2, 128, 8 * (KVLR + DR)], bf16)
    wuk = inp("wuk", [NH, 128, NKV, 128], bf16)
    wuv = inp("wuv", [NH, 128, NKV, DV], bf16)
    ow_blk = inp("ow_blk", [NH, 128, H], bf16)
    gu_blk = inp("gu_blk", [NIT, 128, 2 * NFC * 128], bf16)
    dw_blk = inp("dw_blk", [NIT // 2, 128, 2 * H], bf16)
    cosk = inp("cosk", [128, NTT, DR // 2], f32)
    sink = inp("sink", [128, NTT, DR // 2], f32)
    eye = inp("eye", [128, 128], bf16)
    ones_row = inp("ones_row", [1, 128], bf16)

    out_rows = nc.dram_tensor("out_rows", [NQT, 128, H], f32,
                              kind="ExternalOutput").ap()

    from contextlib import ExitStack
    with tile.TileContext(nc) as tc, ExitStack() as ctx:
        persist = ctx.enter_context(tc.tile_pool(name="persist", bufs=1))

        def pt(shape, dt, tag):
            return persist.tile(list(shape), dt, tag=tag, name=tag)

        eps_sb = pt([128, 1], f32, "eps")
        nc.vector.memset(eps_sb[:], EPS)
        eye_sb = pt([128, 128], bf16, "eye")
        nc.sync.dma_start(out=eye_sb[:], in_=eye[:])
        x_rows_sb = pt([128, NQT, H], f32, "x_rows")
        for qt in range(NQT):
            nc.sync.dma_start(out=x_rows_sb[:, qt, :], in_=x_rows[qt])
        ones_sb = pt([1, 128], bf16, "ones")
        nc.gpsimd.dma_start(out=ones_sb[:], in_=ones_row[:])
        mask_sb = pt([128, 16], bf16, "mask")
        nc.gpsimd.dma_start(out=mask_sb[:], in_=mask16[:])
        cq_sb = pt([128, NQT, NH * DR // 2], bf16, "cq")
        sq_sb = pt([128, NQT, NH * DR // 2], bf16, "sq")
        for qt in range(NQT):
            nc.gpsimd.dma_start(out=cq_sb[:, qt, :], in_=cosq16[qt])
            nc.gpsimd.dma_start(out=sq_sb[:, qt, :], in_=sinq16[qt])

        rstd_all = pt([128, NTT], f32, "rstd_all")
        c_hat = pt([128, NTT, CAUG], bf16, "c_hat")
        # ones column + zero pad for the denominator trick
        nc.vector.memset(c_hat[:, :, 512:513], 1.0)
        nc.vector.memset(c_hat[:, :, 513:CAUG], 0.0)
        kT_lat = pt([128, NKV, T], bf16, "kT_lat")
        kT_rope = pt([64, T], bf16, "kT_rope")
        qcT = pt([128, NRC, RPC], bf16, "qcT")
        qropeT = pt([64, NH, RPC], bf16, "qropeT")
        o_vT = pt([128, NH, RPC], bf16, "o_vT")
        hnT = pt([128, NFC, RPC], bf16, "hnT")
        act_all = pt([128, NIT, RPC], bf16, "act_all")

        # =============== phase B1: row stats + q_a GEMM ===============
        with tc.tile_pool(name="pb", bufs=2) as pb, \
             tc.tile_pool(name="pbs", bufs=1) as pbs, \
             tc.tile_pool(name="pbd", bufs=2) as pbd, \
             tc.tile_pool(name="pbps", bufs=2, space="PSUM") as pbps, \
             tc.tile_pool(name="pbtp", bufs=2, space="PSUM") as pbtp:
            rstd_rows = pbs.tile([128, NQT], f32, name="rstd_rows")
            ssq_r = pbs.tile([128, NQT], f32, name="ssq_r")
            for qt in range(NQT):
                scrap = pbd.tile([128, H], bf16, tag="scrapq", name="scrapq")
                nc.vector.scalar_tensor_tensor(
                    scrap[:], x_rows_sb[:, qt, :], 1.0, x_rows_sb[:, qt, :],
                    ALU.bypass, ALU.mult, accum_out=ssq_r[:, qt:qt + 1])
            nc.scalar.activation(rstd_rows[:], ssq_r[:], AF.Ln,
                                 bias=eps_sb[:], scale=1.0 / H)
            nc.scalar.activation(rstd_rows[:], rstd_rows[:], AF.Exp, scale=-0.5)

            xTc_sb = pbs.tile([128, NFC, RPC], bf16, name="xTc_sb")
            nc.sync.dma_start(out=xTc_sb[:], in_=xTc[:])
            qa_ps = [pbps.tile([128, QLR], f32, tag="qaps", name="qaps")
                     for _ in range(NQT)]
            for fg in range(NFC // 4):
                qaw = pb.tile([128, 4, QLR], bf16, tag="qaw", name="qaw")
                nc.sync.dma_start(out=qaw[:], in_=qa_blk[fg])
                for sub in range(4):
                    fc = 4 * fg + sub
                    for qt in range(NQT):
                        for nn in range(QLR // 512):
                            nc.tensor.matmul(
                                qa_ps[qt][:, nn * 512:(nn + 1) * 512],
                                xTc_sb[:, fc, qt * 128:(qt + 1) * 128],
                                qaw[:, sub, nn * 512:(nn + 1) * 512],
                                start=(fc == 0), stop=(fc == NFC - 1))
            qc = pbs.tile([128, NQT, QLR], bf16, name="qc")
            ssq_q = pbs.tile([128, NQT], f32, name="ssq_q")
            for qt in range(NQT):
                scr = pbd.tile([128, QLR], bf16, tag="scrq2", name="scrq2")
                nc.scalar.activation(scr[:], qa_ps[qt][:], AF.Square,
                                     accum_out=ssq_q[:, qt:qt + 1])
            sq = pbs.tile([128, NQT], f32, name="sqn")
            nc.vector.tensor_mul(sq[:], rstd_rows[:], rstd_rows[:])
            nc.vector.tensor_mul(sq[:], sq[:], ssq_q[:])
            nc.scalar.activation(sq[:], sq[:], AF.Ln, bias=eps_sb[:],
                                 scale=1.0 / QLR)
            nc.scalar.activation(sq[:], sq[:], AF.Exp, scale=-0.5)
            nc.vector.tensor_mul(sq[:], rstd_rows[:], sq[:])
            for qt in range(NQT):
                nc.vector.tensor_scalar_mul(qc[:, qt, :], qa_ps[qt][:],
                                            sq[:, qt:qt + 1])
            for qt in range(NQT):
                for rc in range(NRC):
                    tp = pbtp.tile([128, 128], bf16, tag="tp", name="tp")
                    nc.tensor.transpose(tp[:], qc[:, qt, rc * 128:(rc + 1) * 128],
                                        eye_sb[:])
                    nc.vector.tensor_copy(qcT[:, rc, qt * 128:(qt + 1) * 128],
                                          tp[:])

        # =============== phase B2: rope-q for all heads ===============
        if MAXPH >= 1:
            with tc.tile_pool(name="pr", bufs=2) as pr, \
                 tc.tile_pool(name="prs", bufs=1) as prs, \
                 tc.tile_pool(name="prd", bufs=2) as prd, \
                 tc.tile_pool(name="prps", bufs=2, space="PSUM") as prps, \
                 tc.tile_pool(name="prtp", bufs=2, space="PSUM") as prtp:
                qbr_sb = prs.tile([128, NRC, NH * DR], bf16, name="qbr_sb")
                for rg in range(NRC // 3):
                    nc.sync.dma_start(
                        out=qbr_sb[:, 3 * rg:3 * (rg + 1), :], in_=qbr_blk[rg])
                qr_sb = prs.tile([128, NQT, NH * DR], bf16, name="qr_sb")
                ND = NH * DR
                for qt in range(NQT):
                    rps = prps.tile([128, ND], f32, tag="rps", name="rps")
                    for rc in range(NRC):
                        for half in range(2):
                            nc.tensor.matmul(
                                rps[:, half * 512:(half + 1) * 512],
                                qcT[:, rc, qt * 128:(qt + 1) * 128],
                                qbr_sb[:, rc, half * 512:(half + 1) * 512],
                                start=(rc == 0), stop=(rc == NRC - 1))
                    # rotate pairs: flat col h*64+2i <-> (head h, pair i);
                    # cq/sq are packed h-major [128, NH*32] to match
                    x1 = rps[:, 0:ND:2]
                    x2 = rps[:, 1:ND:2]
                    cs = cq_sb[:, qt, :]
                    sn = sq_sb[:, qt, :]
                    ta = prd.tile([128, ND // 2], f32, tag="ta", name="ta")
                    tb = prd.tile([128, ND // 2], f32, tag="tb", name="tb")
                    nc.vector.tensor_mul(ta[:], x1, cs)
                    nc.vector.tensor_mul(tb[:], x2, sn)
                    nc.vector.tensor_sub(qr_sb[:, qt, 0:ND:2], ta[:], tb[:])
                    nc.vector.tensor_mul(ta[:], x2, cs)
                    nc.vector.tensor_mul(tb[:], x1, sn)
                    nc.vector.tensor_add(qr_sb[:, qt, 1:ND:2], ta[:], tb[:])
                for qt in range(NQT):
                    for h in range(NH):
                        tp = prtp.tile([128, 128], bf16, tag="tp", name="tp")
                        nc.tensor.transpose(
                            tp[0:64, :],
                            qr_sb[:, qt, h * DR:(h + 1) * DR], eye_sb[:])
                        nc.vector.tensor_copy(
                            qropeT[:, h, qt * 128:(qt + 1) * 128], tp[0:64, :])

        # =============== phase 0: all-token stats + kv path ===============
        if MAXPH >= 2:
            with tc.tile_pool(name="p0", bufs=2) as p0, \
                 tc.tile_pool(name="p0x", bufs=2) as p0x, \
                 tc.tile_pool(name="p0w", bufs=1) as p0w, \
                 tc.tile_pool(name="p0s", bufs=1) as p0s, \
                 tc.tile_pool(name="p0d", bufs=2) as p0d, \
                 tc.tile_pool(name="p0ps", bufs=2, space="PSUM") as p0ps, \
                 tc.tile_pool(name="p0tp", bufs=2, space="PSUM") as p0tp:
                cosk_sb = p0s.tile([128, NTT, DR // 2], f32, name="cosk_sb")
                nc.gpsimd.dma_start(out=cosk_sb[:], in_=cosk[:])
                sink_sb = p0s.tile([128, NTT, DR // 2], f32, name="sink_sb")
                nc.gpsimd.dma_start(out=sink_sb[:], in_=sink[:])
                ssq_all = p0s.tile([128, NTT], f32, name="ssq_all")

                kvw = p0w.tile([128, NFC, KVLR + DR], bf16, name="kvw")
                for hf in range(2):
                    nc.sync.dma_start(out=kvw[:, 8 * hf:8 * (hf + 1), :],
                                      in_=kva_blk[hf])
                kr_all = p0s.tile([128, NTT, DR], bf16, name="kr_all")
                ssq_kv = p0s.tile([128, NTT], f32, name="ssq_kv")
                s_ck = p0s.tile([128, NTT], f32, name="s_ck")
                t1 = p0s.tile([128, NTT], f32, name="t1")
                # stage RAW kv outputs (c_hat holds unscaled latent for now;
                # scales applied in one batch below so the GEMM never stalls
                # on the stats chain and Ln/Exp tables load once)
                for ti in range(NTT // 2):
                    xt = p0.tile([128, 2, NFC, 128], bf16, tag="xt", name="xt")
                    nc.sync.dma_start(out=xt[:], in_=xT_blk[ti])
                    for sub in range(2):
                        tt = 2 * ti + sub
                        ps = p0ps.tile([128, 1152], f32, tag="kvps",
                                       name="kvps")
                        for fc in range(NFC):
                            nc.tensor.matmul(ps[:, 0:512], xt[:, sub, fc, :],
                                             kvw[:, fc, 0:512],
                                             start=(fc == 0),
                                             stop=(fc == NFC - 1))
                            nc.tensor.matmul(ps[:, 512:576], xt[:, sub, fc, :],
                                             kvw[:, fc, 512:576],
                                             start=(fc == 0),
                                             stop=(fc == NFC - 1))
                            # gram block in its own PSUM bank: diag = ssq
                            nc.tensor.matmul(ps[:, 1024:1152], xt[:, sub, fc, :],
                                             xt[:, sub, fc, :],
                                             start=(fc == 0),
                                             stop=(fc == NFC - 1))
                        scr2 = p0d.tile([128, KVLR], bf16, tag="scr2",
                                        name="scr2")
                        nc.scalar.activation(scr2[:], ps[:, 0:512], AF.Square,
                                             accum_out=ssq_kv[:, tt:tt + 1])
                        scrg = p0d.tile([128, 128], f32, tag="scrg",
                                        name="scrg")
                        nc.vector.tensor_copy(scrg[:], ps[:, 1024:1152])
                        scrd = p0d.tile([128, 128], f32, tag="scrd",
                                        name="scrd")
                        nc.vector.scalar_tensor_tensor(
                            scrd[:], scrg[:], 1.0, eye_sb[:],
                            ALU.bypass, ALU.mult,
                            accum_out=ssq_all[:, tt:tt + 1])
                        nc.vector.tensor_copy(c_hat[:, tt, 0:512],
                                              ps[:, 0:512])
                        nc.vector.tensor_copy(kr_all[:, tt, :],
                                              ps[:, 512:576])
                nc.scalar.activation(rstd_all[:], ssq_all[:], AF.Ln,
                                     bias=eps_sb[:], scale=1.0 / H)
                nc.scalar.activation(rstd_all[:], rstd_all[:], AF.Exp,
                                     scale=-0.5)
                # batch: s_ck = rstd * rsqrt(mean((rstd*kv)^2)+eps)
                nc.vector.tensor_mul(t1[:], rstd_all[:], rstd_all[:])
                nc.vector.tensor_mul(t1[:], t1[:], ssq_kv[:])
                nc.scalar.activation(t1[:], t1[:], AF.Ln, bias=eps_sb[:],
                                     scale=1.0 / KVLR)
                nc.scalar.activation(t1[:], t1[:], AF.Exp, scale=-0.5)
                nc.vector.tensor_mul(s_ck[:], rstd_all[:], t1[:])
                for tt in range(NTT):
                    nc.vector.tensor_scalar_mul(c_hat[:, tt, 0:512],
                                                c_hat[:, tt, 0:512],
                                                s_ck[:, tt:tt + 1])
                    nc.vector.tensor_scalar_mul(kr_all[:, tt, :],
                                                kr_all[:, tt, :],
                                                rstd_all[:, tt:tt + 1])
                # batched k-rope rotation
                krf = p0s.tile([128, NTT, DR], bf16, name="krf")
                x1 = kr_all[:, :, 0:DR:2]
                x2 = kr_all[:, :, 1:DR:2]
                ta = p0s.tile([128, NTT, DR // 2], f32, name="tak")
                tb = p0s.tile([128, NTT, DR // 2], f32, name="tbk")
                nc.vector.tensor_mul(ta[:], x1, cosk_sb[:])
                nc.vector.tensor_mul(tb[:], x2, sink_sb[:])
                nc.vector.tensor_sub(krf[:, :, 0:DR:2], ta[:], tb[:])
                nc.vector.tensor_mul(ta[:], x2, cosk_sb[:])
                nc.vector.tensor_mul(tb[:], x1, sink_sb[:])
                nc.vector.tensor_add(krf[:, :, 1:DR:2], ta[:], tb[:])
                for tt in range(NTT):
                    for rc in range(NKV):
                        tp = p0tp.tile([128, 128], bf16, tag="tp", name="tp")
                        nc.tensor.transpose(
                            tp[:], c_hat[:, tt, rc * 128:(rc + 1) * 128],
                            eye_sb[:])
                        nc.vector.tensor_copy(
                            kT_lat[:, rc, tt * 128:(tt + 1) * 128], tp[:])
                    tp = p0tp.tile([128, 128], bf16, tag="tp", name="tp")
                    nc.tensor.transpose(tp[0:64, :], krf[:, tt, :], eye_sb[:])
                    nc.vector.tensor_copy(kT_rope[:, tt * 128:(tt + 1) * 128],
                                          tp[0:64, :])

        # =============== phase 2: attention (all in T layouts) ===============
        if MAXPH >= 3:
            with tc.tile_pool(name="p2", bufs=2) as p2, \
                 tc.tile_pool(name="p2d", bufs=2) as p2d, \
                 tc.tile_pool(name="p2e", bufs=2) as p2e, \
                 tc.tile_pool(name="p2ps", bufs=3, space="PSUM") as p2ps, \
                 tc.tile_pool(name="olps", bufs=4, space="PSUM") as olps, \
                 tc.tile_pool(name="denps", bufs=1, space="PSUM") as denps:
                for h in range(NH):
                    qbw = p2.tile([128, NRC * DN], bf16, tag="qbw", name="qbw")
                    nc.gpsimd.dma_start(out=qbw[:], in_=qbn_blk[h])
                    wuk_h = p2.tile([128, NKV, 128], bf16, tag="wuk_h",
                                    name="wuk_h")
                    nc.gpsimd.dma_start(out=wuk_h[:], in_=wuk[h])
                    wuv_h = p2.tile([128, NKV, DV], bf16, tag="wuv_h",
                                    name="wuv_h")
                    nc.gpsimd.dma_start(out=wuv_h[:], in_=wuv[h])

                    # qnT[dn, q] = sum_rc qb_nope[rc].T @ qcT[rc]
                    qn_ps = p2ps.tile([128, RPC], f32, tag="sp", name="qnps")
                    for rc in range(NRC):
                        nc.tensor.matmul(qn_ps[:], qbw[:, rc * DN:(rc + 1) * DN],
                                         qcT[:, rc, :],
                                         start=(rc == 0), stop=(rc == NRC - 1))
                    qnT = p2d.tile([128, RPC], bf16, tag="qnT", name="qnT")
                    nc.scalar.copy(qnT[:], qn_ps[:])
                    # qT_lat[lat, q] = wuk.T @ qnT  (4 latent chunks)
                    qlat = p2d.tile([128, NKV, RPC], bf16, tag="qlat",
                                    name="qlat")
                    for rc in range(NKV):
                        lp = p2ps.tile([128, RPC], f32, tag="sp", name="latps")
                        nc.tensor.matmul(lp[:], wuk_h[:, rc, :], qnT[:],
                                         start=True, stop=True)
                        nc.scalar.copy(qlat[:, rc, :], lp[:])

                    ol = [olps.tile([128, RPC], f32, tag="ol", name="ol")
                          for _ in range(NKV)]
                    den = denps.tile([8, RPC], f32, tag="den", name="den")
                    eT = p2e.tile([128, NTT, RPC], bf16, tag="eT", name="eT")
                    for kt in range(NTT):
                        w0 = 16 * kt
                        sp = p2ps.tile([128, RPC], f32, tag="sp", name="sp")
                        for dc in range(NKV):
                            nc.tensor.matmul(
                                sp[:, w0:], kT_lat[:, dc, kt * 128:(kt + 1) * 128],
                                qlat[:, dc, w0:], start=(dc == 0), stop=False)
                        nc.tensor.matmul(
                            sp[:, w0:], kT_rope[:, kt * 128:(kt + 1) * 128],
                            qropeT[:, h, w0:], start=False, stop=True)
                        nc.scalar.activation(eT[:, kt, w0:], sp[:, w0:], AF.Exp)
                        nc.vector.tensor_mul(eT[:, kt, w0:w0 + 16],
                                             eT[:, kt, w0:w0 + 16], mask_sb[:])
                    for kt in range(NTT):
                        w0 = 16 * kt
                        for rc in range(NKV):
                            nc.tensor.matmul(
                                ol[rc][:, w0:],
                                c_hat[:, kt, rc * 128:(rc + 1) * 128],
                                eT[:, kt, w0:],
                                start=(kt == 0), stop=(kt == NTT - 1))
                        nc.tensor.matmul(
                            den[:, w0:], c_hat[:, kt, 512:CAUG], eT[:, kt, w0:],
                            start=(kt == 0), stop=(kt == NTT - 1))
                    rinv = p2d.tile([1, RPC], bf16, tag="rinv", name="rinv")
                    with nc.allow_low_precision(
                            reason="1/den feeds bf16 matmul rhs anyway"):
                        nc.vector.reciprocal(rinv[:], den[0:1, :])
                    rb_ps = p2ps.tile([128, RPC], f32, tag="sp", name="rbps")
                    nc.tensor.matmul(rb_ps[:], ones_sb[:], rinv[:],
                                     start=True, stop=True)
                    rb = p2d.tile([128, RPC], bf16, tag="rb", name="rb")
                    nc.scalar.copy(rb[:], rb_ps[:])
                    olT = p2d.tile([128, NKV, RPC], bf16, tag="olT", name="olT")
                    for rc in range(NKV):
                        nc.vector.tensor_copy(olT[:, rc, :], ol[rc][:])
                    ovp = p2ps.tile([128, RPC], f32, tag="sp", name="ovp")
                    for rc in range(NKV):
                        nc.tensor.matmul(ovp[:], wuv_h[:, rc, :], olT[:, rc, :],
                                         start=(rc == 0), stop=(rc == NKV - 1))
                    nc.vector.tensor_mul(o_vT[:, h, :], ovp[:], rb[:])

        # =============== phase 3: o_proj + residual + post-norm ===============
        if MAXPH >= 4:
            with tc.tile_pool(name="p3", bufs=4) as p3, \
                 tc.tile_pool(name="p3s", bufs=1) as p3s, \
                 tc.tile_pool(name="p3d", bufs=2) as p3d:
                hn = p3s.tile([128, NQT, H], bf16, name="hn")
                with tc.tile_pool(name="p3ps", bufs=2, space="PSUM") as p3ps:
                    op_ps = [p3ps.tile([128, H], f32, tag="opps", name="opps")
                             for _ in range(NQT)]
                    for hc in range(NH):
                        oww = p3.tile([128, H], bf16, tag="oww", name="oww")
                        nc.sync.dma_start(out=oww[:], in_=ow_blk[hc])
                        for qt in range(NQT):
                            for nn in range(4):
                                nc.tensor.matmul(
                                    op_ps[qt][:, nn * 512:(nn + 1) * 512],
                                    o_vT[:, hc, qt * 128:(qt + 1) * 128],
                                    oww[:, nn * 512:(nn + 1) * 512],
                                    start=(hc == 0), stop=(hc == NH - 1))
                    ssq2 = p3s.tile([128, NQT], f32, name="ssq2")
                    for qt in range(NQT):
                        nc.vector.tensor_add(x_rows_sb[:, qt, :],
                                             x_rows_sb[:, qt, :], op_ps[qt][:])
                    for qt in range(NQT):
                        scr = p3d.tile([128, H], bf16, tag="scr3", name="scr3")
                        nc.vector.scalar_tensor_tensor(
                            scr[:], x_rows_sb[:, qt, :], 1.0,
                            x_rows_sb[:, qt, :],
                            ALU.bypass, ALU.mult, accum_out=ssq2[:, qt:qt + 1])
                    nc.scalar.activation(ssq2[:], ssq2[:], AF.Ln,
                                         bias=eps_sb[:], scale=1.0 / H)
                    nc.scalar.activation(ssq2[:], ssq2[:], AF.Exp, scale=-0.5)
                    for qt in range(NQT):
                        nc.vector.tensor_scalar_mul(hn[:, qt, :],
                                                    x_rows_sb[:, qt, :],
                                                    ssq2[:, qt:qt + 1])
                with tc.tile_pool(name="p3tp", bufs=2, space="PSUM") as p3tp:
                    for qt in range(NQT):
                        for fc in range(NFC):
                            tp = p3tp.tile([128, 128], bf16, tag="tp",
                                           name="tp")
                            nc.tensor.transpose(
                                tp[:], hn[:, qt, fc * 128:(fc + 1) * 128],
                                eye_sb[:])
                            nc.vector.tensor_copy(
                                hnT[:, fc, qt * 128:(qt + 1) * 128], tp[:])

        # =============== phase 4: MLP ===============
        if MAXPH >= 5:
            with tc.tile_pool(name="p4", bufs=3) as p4, \
                 tc.tile_pool(name="p4ps", bufs=2, space="PSUM") as p4ps:
                GOFF = NFC * 128
                for it in range(NIT):
                    gw = p4.tile([128, 2 * NFC * 128], bf16, tag="gw", name="gw")
                    nc.sync.dma_start(out=gw[:], in_=gu_blk[it])
                    gp = p4ps.tile([128, RPC], f32, tag="gp", name="gp")
                    up = p4ps.tile([128, RPC], f32, tag="up", name="up")
                    for fc in range(NFC):
                        nc.tensor.matmul(gp[:], gw[:, fc * 128:(fc + 1) * 128],
                                         hnT[:, fc, :],
                                         start=(fc == 0), stop=(fc == NFC - 1))
                        nc.tensor.matmul(up[:],
                                         gw[:, GOFF + fc * 128:GOFF + (fc + 1) * 128],
                                         hnT[:, fc, :],
                                         start=(fc == 0), stop=(fc == NFC - 1))
                    gs = p4.tile([128, RPC], bf16, tag="gs", name="gs")
                    nc.scalar.activation(gs[:], gp[:], AF.Silu)
                    nc.vector.tensor_mul(act_all[:, it, :], gs[:], up[:])
            with tc.tile_pool(name="p4b", bufs=3) as p4b, \
                 tc.tile_pool(name="p4s", bufs=2) as p4s, \
                 tc.tile_pool(name="p4bps", bufs=2, space="PSUM") as p4bps:
                o_ps = [p4bps.tile([128, H], f32, tag="ops", name="ops")
                        for _ in range(NQT)]
                for jt in range(NIT // 2):
                    dw = p4b.tile([128, 2 * H], bf16, tag="dw", name="dw")
                    nc.sync.dma_start(out=dw[:], in_=dw_blk[jt])
                    for k in range(2):
                        it = 2 * jt + k
                        for qt in range(NQT):
                            for nn in range(4):
                                nc.tensor.matmul(
                                    o_ps[qt][:, nn * 512:(nn + 1) * 512],
                                    act_all[:, it, qt * 128:(qt + 1) * 128],
                                    dw[:, k * H + nn * 512:k * H + (nn + 1) * 512],
                                    start=(it == 0), stop=(it == NIT - 1))
                for qt in range(NQT):
                    fin = p4s.tile([128, H], f32, tag="fin", name="fin")
                    nc.vector.tensor_add(fin[:], x_rows_sb[:, qt, :],
                                         o_ps[qt][:])
                    nc.sync.dma_start(out=out_rows[qt], in_=fin[:])

        if MAXPH < 5:
            with tc.tile_pool(name="pex", bufs=2) as pex:
                for qt in range(NQT):
                    fin = pex.tile([128, H], f32, tag="finx", name="finx")
                    nc.vector.tensor_copy(fin[:], x_rows_sb[:, qt, :])
                    nc.sync.dma_start(out=out_rows[qt], in_=fin[:])
    nc.compile()
    return nc


def _host_prep(inputs):
    f32 = np.float32
    bf = bfloat16
    x = np.asarray(inputs["hidden_states"], f32)
    pos = np.asarray(inputs["positions"]).astype(f32)

    lnw_in = np.asarray(inputs["input_ln_w"], f32)
    q_a_w = np.asarray(inputs["q_a_w"], f32) * lnw_in[:, None]
    kv_a_w = np.asarray(inputs["kv_a_w"], f32) * lnw_in[:, None]
    q_b_w = (np.asarray(inputs["q_b_w"], f32)
             * np.asarray(inputs["q_a_ln_w"], f32)[:, None]) * SCALE
    kvln = np.asarray(inputs["kv_a_ln_w"], f32)
    w_uk = np.asarray(inputs["w_uk"], f32) * kvln[:, None, None]
    w_uv = np.asarray(inputs["w_uv"], f32) * kvln[:, None, None]
    o_w = np.asarray(inputs["o_w"], f32)
    pln = np.asarray(inputs["post_ln_w"], f32)
    gate_w = np.asarray(inputs["gate_w"], f32) * pln[:, None]
    up_w = np.asarray(inputs["up_w"], f32) * pln[:, None]
    down_w = np.asarray(inputs["down_w"], f32)

    xT = np.ascontiguousarray(x.T)
    inv_freq = 1.0 / (THETA ** (np.arange(0, DR, 2, dtype=f32) / DR))
    ang = pos[:, None] * inv_freq
    cos_t = np.cos(ang).astype(f32)
    sin_t = np.sin(ang).astype(f32)

    gu = np.zeros((2, IPAD, H), f32)
    gu[0, :INTER] = gate_w.T
    gu[1, :INTER] = up_w.T

    qb4 = q_b_w.astype(bf).reshape(NRC, 128, NH, DN + DR)

    rep = {
        "xT_blk": np.ascontiguousarray(
            xT.astype(bf).reshape(NFC, 128, NTT, 128).transpose(2, 1, 0, 3)
            .reshape(NTT // 2, 2, 128, NFC, 128).transpose(0, 2, 1, 3, 4)),
        "qa_blk": np.ascontiguousarray(
            q_a_w.astype(bf).reshape(NFC // 4, 4, 128, QLR)
            .transpose(0, 2, 1, 3).reshape(NFC // 4, 128, 4 * QLR)),
        # qbn_blk[h, p, rc*DN+d] = SCALE*q_b_w[rc*128+p, h*192+d], d < DN
        "qbn_blk": np.ascontiguousarray(
            qb4[:, :, :, :DN].transpose(2, 1, 0, 3).reshape(NH, 128, NRC * DN)),
        # qbr_blk[rc, p, h*64+j] = SCALE*q_b_w[rc*128+p, h*192+128+j]
        "qbr_blk": np.ascontiguousarray(
            qb4[:, :, :, DN:].reshape(NRC // 3, 3, 128, NH * DR)
            .transpose(0, 2, 1, 3).reshape(NRC // 3, 128, 3 * NH * DR)),
        "kva_blk": np.ascontiguousarray(
            kv_a_w.astype(bf).reshape(2, 8, 128, KVLR + DR)
            .transpose(0, 2, 1, 3).reshape(2, 128, 8 * (KVLR + DR))),
        "wuk": np.ascontiguousarray(
            w_uk.transpose(1, 2, 0).reshape(NH, 128, NKV, 128).astype(bf)),
        "wuv": np.ascontiguousarray(
            w_uv.transpose(1, 0, 2).reshape(NH, NKV, 128, DV)
            .transpose(0, 2, 1, 3).astype(bf)),
        "ow_blk": np.ascontiguousarray(o_w.astype(bf).reshape(NH, 128, H)),
        # gu_blk[it, hp, g*NFC*128 + fc*128 + ip] = gate/up[fc*128+hp, it*128+ip]
        "gu_blk": np.ascontiguousarray(
            gu.reshape(2, NIT, 128, NFC, 128).transpose(1, 4, 0, 3, 2)
            .reshape(NIT, 128, 2 * NFC * 128).astype(bf)),
        "dw_blk": np.ascontiguousarray(
            np.concatenate([down_w, np.zeros((IPAD - INTER, H), f32)], 0)
            .astype(bf).reshape(NIT // 2, 2, 128, H).transpose(0, 2, 1, 3)
            .reshape(NIT // 2, 128, 2 * H)),
        "cosk": np.ascontiguousarray(
            cos_t.reshape(NTT, 128, DR // 2).transpose(1, 0, 2)),
        "sink": np.ascontiguousarray(
            sin_t.reshape(NTT, 128, DR // 2).transpose(1, 0, 2)),
        "eye": np.eye(128, dtype=bf),
        "ones_row": np.ones((1, 128), bf),
    }

    per_core = []
    kpos = np.arange(128)
    qpos = np.arange(16)
    for c in range(NCORES):
        rows = np.arange(c, T, NCORES)
        m = dict(rep)
        m["x_rows"] = np.ascontiguousarray(x[rows].reshape(NQT, 128, H))
        m["xTc"] = np.ascontiguousarray(
            xT[:, rows].astype(bf).reshape(NFC, 128, RPC)
            .transpose(1, 0, 2).reshape(128, NFC * RPC))
        cq = np.broadcast_to(
            cos_t[rows].reshape(NQT, 128, 1, DR // 2),
            (NQT, 128, NH, DR // 2))
        sq = np.broadcast_to(
            sin_t[rows].reshape(NQT, 128, 1, DR // 2),
            (NQT, 128, NH, DR // 2))
        m["cosq16"] = np.ascontiguousarray(
            cq.reshape(NQT, 128, NH * DR // 2).astype(bf))
        m["sinq16"] = np.ascontiguousarray(
            sq.reshape(NQT, 128, NH * DR // 2).astype(bf))
        # mask16[j, i] = key j attends-allowed for query row (16*kt + i):
        # key pos 128*kt + j <= query pos 8*(16*kt+i) + c  <=>  j <= 8*i + c
        m["mask16"] = np.ascontiguousarray(
            (kpos[:, None] <= 8 * qpos[None, :] + c).astype(bf))
        per_core.append(m)
    return per_core


def kernel(**inputs):
    from concourse import bass_utils

    if "nc" not in _CACHE:
        _CACHE["nc"] = _build_module()
    nc = _CACHE["nc"]

    import os
    in_maps = _host_prep(inputs)
    trace = bool(os.environ.get("BASS_KERNEL_TRACE"))
    res = bass_utils.run_bass_kernel_spmd(nc, in_maps,
                                          core_ids=list(range(NCORES)),
                                          trace=trace)
    if trace and res.exec_time_ns is not None:
        print(f"HW exec time: {res.exec_time_ns} ns")
        _CACHE["last_result"] = res
    out = np.zeros((T, H), np.float32)
    for c in range(NCORES):
        rows = np.arange(c, T, NCORES)
        out[rows] = res.results[c]["out_rows"].reshape(RPC, H)
    return out


# revision 21
# speedup vs baseline: 1.3833x; 1.0232x over previous
"""DeepseekV3 decoder layer (MLA + SwiGLU MLP), T=2048 prefill, fp32 I/O.

Sharding: sequence-parallel striped - core c owns token rows c::8 (256 rows),
so all 8 cores run one identical SPMD program with balanced causal work.
The KV latent path (all 2048 tokens) is replicated on every core; outputs
are disjoint row sets concatenated on host.

v2 redesign vs v1:
- Causal tightening: with striped rows, key tile kt is attended only by
  query rows >= 16*kt on EVERY core, so score/attn matmuls shrink their
  free dim per key tile (2176 vs 3072 q-cols of work) and masking reduces
  to one [128,16] triangle applied to the first 16 active columns.
- Attention computed fully in transposed layouts: qnT/qT_lat directly from
  qcT (no per-head transposes), o_latent^T accumulated directly via
  lhsT=c_hat tiles, denominator via an appended ones-column in c_hat,
  1/den applied to o_vT via a K=1 broadcast matmul.
- SCALE folded into q_b weights on host; LN weights folded as in v1.
- MLP/o_proj weights streamed on the gpsimd (SWDGE) queue so the big
  weight stream never head-of-line-blocks latency-critical sync-queue
  loads; gate+up fused to 1MB DMAs, down proj 1MB (2 tiles per DMA).
"""

import numpy as np
import ml_dtypes

bfloat16 = ml_dtypes.bfloat16

T = 2048
H = 2048
NH = 16
QLR = 1536
KVLR = 512
DN = 128
DR = 64
DV = 128
INTER = 10944
NCORES = 8
RPC = T // NCORES          # 256 query rows per core
NQT = RPC // 128           # 2
NTT = T // 128             # 16
NFC = H // 128             # 16
NRC = QLR // 128           # 12
NKV = KVLR // 128          # 4
NIT = 86
IPAD = NIT * 128
EPS = 1e-6
SCALE = (DN + DR) ** -0.5
THETA = 10000.0
CAUG = 520                 # c_hat free width: 512 latent + 1 ones + 7 pad

_CACHE = {}


def _build_module():
    import os
    MAXPH = int(os.environ.get("KERNEL_MAXPH", "9"))
    import concourse.bass as bass
    import concourse.tile as tile
    from concourse import bacc, mybir

    f32 = mybir.dt.float32
    bf16 = mybir.dt.bfloat16
    AF = mybir.ActivationFunctionType
    ALU = mybir.AluOpType

    nc = bacc.Bacc("TRN2", target_bir_lowering=False, debug=False,
                   enable_asserts=False, num_devices=NCORES)

    def inp(name, shape, dt):
        return nc.dram_tensor(name, list(shape), dt, kind="ExternalInput").ap()

    # per-core inputs
    x_rows = inp("x_rows", [NQT, 128, H], f32)
    xTc = inp("xTc", [128, NFC * RPC], bf16)
    cosq16 = inp("cosq16", [NQT, 128, NH * DR // 2], bf16)
    sinq16 = inp("sinq16", [NQT, 128, NH * DR // 2], bf16)
    mask16 = inp("mask16", [128, 16], bf16)
    # replicated inputs
    xT_blk = inp("xT_blk", [NTT // 2, 128, 2, NFC, 128], bf16)
    qa_blk = inp("qa_blk", [NFC // 4, 128, 4 * QLR], bf16)
    qbn_blk = inp("qbn_blk", [NH, 128, NRC * DN], bf16)
    qbr_blk = inp("qbr_blk", [NRC // 3, 128, 3 * NH * DR], bf16)
    kva_blk = inp("kva_blk", [2, 128, 8 * (KVLR + DR)], bf16)
    wuk = inp("wuk", [NH, 128, NKV, 128], bf16)
    wuv = inp("wuv", [NH, 128, NKV, DV], bf16)
    ow_blk = inp("ow_blk", [NH, 128, H], bf16)
    gu_blk = inp("gu_blk", [NIT, 128, 2 * NFC * 128], bf16)
    dw_blk = inp("dw_blk", [NIT // 2, 128, 2 * H], bf16)
    cosk = inp("cosk", [128, NTT, DR // 2], f32)
    sink = inp("sink", [128, NTT, DR // 2], f32)
    eye = inp("eye", [128, 128], bf16)
    ones_row = inp("ones_row", [1, 128], bf16)

    out_rows = nc.dram_tensor("out_rows", [NQT, 128, H], f32,
                              kind="ExternalOutput").ap()

    from contextlib import ExitStack
    with tile.TileContext(nc) as tc, ExitStack() as ctx:
        persist = ctx.enter_context(tc.tile_pool(name="persist", bufs=1))

        def pt(shape, dt, tag):
            return persist.tile(list(shape), dt, tag=tag, name=tag)

        eps_sb = pt([128, 1], f32, "eps")
        nc.vector.memset(eps_sb[:], EPS)
        eye_sb = pt([128, 128], bf16, "eye")
        nc.sync.dma_start(out=eye_sb[:], in_=eye[:])
        x_rows_sb = pt([128, NQT, H], f32, "x_rows")
        for qt in range(NQT):
            nc.gpsimd.dma_start(out=x_rows_sb[:, qt, :], in_=x_rows[qt])
        ones_sb = pt([1, 128], bf16, "ones")
        nc.gpsimd.dma_start(out=ones_sb[:], in_=ones_row[:])
        mask_sb = pt([128, 16], bf16, "mask")
        nc.gpsimd.dma_start(out=mask_sb[:], in_=mask16[:])
        cq_sb = pt([128, NQT, NH * DR // 2], bf16, "cq")
        sq_sb = pt([128, NQT, NH * DR // 2], bf16, "sq")
        for qt in range(NQT):
            nc.gpsimd.dma_start(out=cq_sb[:, qt, :], in_=cosq16[qt])
            nc.gpsimd.dma_start(out=sq_sb[:, qt, :], in_=sinq16[qt])

        rstd_all = pt([128, NTT], f32, "rstd_all")
        c_hat = pt([128, NTT, CAUG], bf16, "c_hat")
        # ones column + zero pad for the denominator trick
        nc.vector.memset(c_hat[:, :, 512:513], 1.0)
        nc.vector.memset(c_hat[:, :, 513:CAUG], 0.0)
        kT_lat = pt([128, NKV, T], bf16, "kT_lat")
        kT_rope = pt([64, T], bf16, "kT_rope")
        qcT = pt([128, NRC, RPC], bf16, "qcT")
        qropeT = pt([64, NH, RPC], bf16, "qropeT")
        o_vT = pt([128, NH, RPC], bf16, "o_vT")
        hnT = pt([128, NFC, RPC], bf16, "hnT")
        act_all = pt([128, NIT, RPC], bf16, "act_all")

        # =============== phase B1: row stats + q_a GEMM ===============
        with tc.tile_pool(name="pb", bufs=2) as pb, \
             tc.tile_pool(name="pbs", bufs=1) as pbs, \
             tc.tile_pool(name="pbd", bufs=2) as pbd, \
             tc.tile_pool(name="pbps", bufs=2, space="PSUM") as pbps, \
             tc.tile_pool(name="pbtp", bufs=2, space="PSUM") as pbtp:
            rstd_rows = pbs.tile([128, NQT], f32, name="rstd_rows")
            ssq_r = pbs.tile([128, NQT], f32, name="ssq_r")
            for qt in range(NQT):
                scrap = pbd.tile([128, H], bf16, tag="scrapq", name="scrapq")
                nc.vector.scalar_tensor_tensor(
                    scrap[:], x_rows_sb[:, qt, :], 1.0, x_rows_sb[:, qt, :],
                    ALU.bypass, ALU.mult, accum_out=ssq_r[:, qt:qt + 1])
            nc.scalar.activation(rstd_rows[:], ssq_r[:], AF.Ln,
                                 bias=eps_sb[:], scale=1.0 / H)
            nc.scalar.activation(rstd_rows[:], rstd_rows[:], AF.Exp, scale=-0.5)

            xTc_sb = pbs.tile([128, NFC, RPC], bf16, name="xTc_sb")
            nc.sync.dma_start(out=xTc_sb[:], in_=xTc[:])
            qa_ps = [pbps.tile([128, QLR], f32, tag="qaps", name="qaps")
                     for _ in range(NQT)]
            for fg in range(NFC // 4):
                qaw = pb.tile([128, 4, QLR], bf16, tag="qaw", name="qaw")
                nc.sync.dma_start(out=qaw[:], in_=qa_blk[fg])
                for sub in range(4):
                    fc = 4 * fg + sub
                    for qt in range(NQT):
                        for nn in range(QLR // 512):
                            nc.tensor.matmul(
                                qa_ps[qt][:, nn * 512:(nn + 1) * 512],
                                xTc_sb[:, fc, qt * 128:(qt + 1) * 128],
                                qaw[:, sub, nn * 512:(nn + 1) * 512],
                                start=(fc == 0), stop=(fc == NFC - 1))
            qc = pbs.tile([128, NQT, QLR], bf16, name="qc")
            ssq_q = pbs.tile([128, NQT], f32, name="ssq_q")
            for qt in range(NQT):
                scr = pbd.tile([128, QLR], bf16, tag="scrq2", name="scrq2")
                nc.scalar.activation(scr[:], qa_ps[qt][:], AF.Square,
                                     accum_out=ssq_q[:, qt:qt + 1])
            sq = pbs.tile([128, NQT], f32, name="sqn")
            nc.vector.tensor_mul(sq[:], rstd_rows[:], rstd_rows[:])
            nc.vector.tensor_mul(sq[:], sq[:], ssq_q[:])
            nc.scalar.activation(sq[:], sq[:], AF.Ln, bias=eps_sb[:],
                                 scale=1.0 / QLR)
            nc.scalar.activation(sq[:], sq[:], AF.Exp, scale=-0.5)
            nc.vector.tensor_mul(sq[:], rstd_rows[:], sq[:])
            for qt in range(NQT):
                nc.vector.tensor_scalar_mul(qc[:, qt, :], qa_ps[qt][:],
                                            sq[:, qt:qt + 1])
            for qt in range(NQT):
                for rc in range(NRC):
                    tp = pbtp.tile([128, 128], bf16, tag="tp", name="tp")
                    nc.tensor.transpose(tp[:], qc[:, qt, rc * 128:(rc + 1) * 128],
                                        eye_sb[:])
                    nc.vector.tensor_copy(qcT[:, rc, qt * 128:(qt + 1) * 128],
                                          tp[:])

        # =============== phase B2: rope-q for all heads ===============
        if MAXPH >= 1:
            with tc.tile_pool(name="pr", bufs=2) as pr, \
                 tc.tile_pool(name="prs", bufs=1) as prs, \
                 tc.tile_pool(name="prd", bufs=2) as prd, \
                 tc.tile_pool(name="prps", bufs=2, space="PSUM") as prps, \
                 tc.tile_pool(name="prtp", bufs=2, space="PSUM") as prtp:
                qbr_sb = prs.tile([128, NRC, NH * DR], bf16, name="qbr_sb")
                for rg in range(NRC // 3):
                    nc.sync.dma_start(
                        out=qbr_sb[:, 3 * rg:3 * (rg + 1), :], in_=qbr_blk[rg])
                qr_sb = prs.tile([128, NQT, NH * DR], bf16, name="qr_sb")
                ND = NH * DR
                for qt in range(NQT):
                    rps = prps.tile([128, ND], f32, tag="rps", name="rps")
                    for rc in range(NRC):
                        for half in range(2):
                            nc.tensor.matmul(
                                rps[:, half * 512:(half + 1) * 512],
                                qcT[:, rc, qt * 128:(qt + 1) * 128],
                                qbr_sb[:, rc, half * 512:(half + 1) * 512],
                                start=(rc == 0), stop=(rc == NRC - 1))
                    # rotate pairs: flat col h*64+2i <-> (head h, pair i);
                    # cq/sq are packed h-major [128, NH*32] to match
                    x1 = rps[:, 0:ND:2]
                    x2 = rps[:, 1:ND:2]
                    cs = cq_sb[:, qt, :]
                    sn = sq_sb[:, qt, :]
                    ta = prd.tile([128, ND // 2], f32, tag="ta", name="ta")
                    tb = prd.tile([128, ND // 2], f32, tag="tb", name="tb")
                    nc.vector.tensor_mul(ta[:], x1, cs)
                    nc.vector.tensor_mul(tb[:], x2, sn)
                    nc.vector.tensor_sub(qr_sb[:, qt, 0:ND:2], ta[:], tb[:])
                    nc.vector.tensor_mul(ta[:], x2, cs)
                    nc.vector.tensor_mul(tb[:], x1, sn)
                    nc.vector.tensor_add(qr_sb[:, qt, 1:ND:2], ta[:], tb[:])
                for qt in range(NQT):
                    for h in range(NH):
                        tp = prtp.tile([128, 128], bf16, tag="tp", name="tp")
                        nc.tensor.transpose(
                            tp[0:64, :],
                            qr_sb[:, qt, h * DR:(h + 1) * DR], eye_sb[:])
                        nc.vector.tensor_copy(
                            qropeT[:, h, qt * 128:(qt + 1) * 128], tp[0:64, :])

        # =============== phase 0: all-token stats + kv path ===============
        if MAXPH >= 2:
            with tc.tile_pool(name="p0", bufs=2) as p0, \
                 tc.tile_pool(name="p0x", bufs=2) as p0x, \
                 tc.tile_pool(name="p0w", bufs=1) as p0w, \
                 tc.tile_pool(name="p0s", bufs=1) as p0s, \
                 tc.tile_pool(name="p0d", bufs=2) as p0d, \
                 tc.tile_pool(name="p0ps", bufs=2, space="PSUM") as p0ps, \
                 tc.tile_pool(name="p0tp", bufs=2, space="PSUM") as p0tp:
                cosk_sb = p0s.tile([128, NTT, DR // 2], f32, name="cosk_sb")
                nc.gpsimd.dma_start(out=cosk_sb[:], in_=cosk[:])
                sink_sb = p0s.tile([128, NTT, DR // 2], f32, name="sink_sb")
                nc.gpsimd.dma_start(out=sink_sb[:], in_=sink[:])
                ssq_all = p0s.tile([128, NTT], f32, name="ssq_all")

                kvw = p0w.tile([128, NFC, KVLR + DR], bf16, name="kvw")
                for hf in range(2):
                    nc.sync.dma_start(out=kvw[:, 8 * hf:8 * (hf + 1), :],
                                      in_=kva_blk[hf])
                kr_all = p0s.tile([128, NTT, DR], bf16, name="kr_all")
                ssq_kv = p0s.tile([128, NTT], f32, name="ssq_kv")
                s_ck = p0s.tile([128, NTT], f32, name="s_ck")
                t1 = p0s.tile([128, NTT], f32, name="t1")
                # stage RAW kv outputs (c_hat holds unscaled latent for now;
                # scales applied in one batch below so the GEMM never stalls
                # on the stats chain and Ln/Exp tables load once)
                for ti in range(NTT // 2):
                    xt = p0.tile([128, 2, NFC, 128], bf16, tag="xt", name="xt")
                    nc.sync.dma_start(out=xt[:], in_=xT_blk[ti])
                    for sub in range(2):
                        tt = 2 * ti + sub
                        ps = p0ps.tile([128, 1152], f32, tag="kvps",
                                       name="kvps")
                        for fc in range(NFC):
                            nc.tensor.matmul(ps[:, 0:512], xt[:, sub, fc, :],
                                             kvw[:, fc, 0:512],
                                             start=(fc == 0),
                                             stop=(fc == NFC - 1))
                            nc.tensor.matmul(ps[:, 512:576], xt[:, sub, fc, :],
                                             kvw[:, fc, 512:576],
                                             start=(fc == 0),
                                             stop=(fc == NFC - 1))
                            # gram block in its own PSUM bank: diag = ssq
                            nc.tensor.matmul(ps[:, 1024:1152], xt[:, sub, fc, :],
                                             xt[:, sub, fc, :],
                                             start=(fc == 0),
                                             stop=(fc == NFC - 1))
                        scr2 = p0d.tile([128, KVLR], bf16, tag="scr2",
                                        name="scr2")
                        nc.scalar.activation(scr2[:], ps[:, 0:512], AF.Square,
                                             accum_out=ssq_kv[:, tt:tt + 1])
                        scrg = p0d.tile([128, 128], f32, tag="scrg",
                                        name="scrg")
                        nc.vector.tensor_copy(scrg[:], ps[:, 1024:1152])
                        scrd = p0d.tile([128, 128], f32, tag="scrd",
                                        name="scrd")
                        nc.vector.scalar_tensor_tensor(
                            scrd[:], scrg[:], 1.0, eye_sb[:],
                            ALU.bypass, ALU.mult,
                            accum_out=ssq_all[:, tt:tt + 1])
                        nc.vector.tensor_copy(c_hat[:, tt, 0:512],
                                              ps[:, 0:512])
                        nc.vector.tensor_copy(kr_all[:, tt, :],
                                              ps[:, 512:576])
                nc.scalar.activation(rstd_all[:], ssq_all[:], AF.Ln,
                                     bias=eps_sb[:], scale=1.0 / H)
                nc.scalar.activation(rstd_all[:], rstd_all[:], AF.Exp,
                                     scale=-0.5)
                # batch: s_ck = rstd * rsqrt(mean((rstd*kv)^2)+eps)
                nc.vector.tensor_mul(t1[:], rstd_all[:], rstd_all[:])
                nc.vector.tensor_mul(t1[:], t1[:], ssq_kv[:])
                nc.scalar.activation(t1[:], t1[:], AF.Ln, bias=eps_sb[:],
                                     scale=1.0 / KVLR)
                nc.scalar.activation(t1[:], t1[:], AF.Exp, scale=-0.5)
                nc.vector.tensor_mul(s_ck[:], rstd_all[:], t1[:])
                for tt in range(NTT):
                    nc.vector.tensor_scalar_mul(c_hat[:, tt, 0:512],
                                                c_hat[:, tt, 0:512],
                                                s_ck[:, tt:tt + 1])
                    nc.vector.tensor_scalar_mul(kr_all[:, tt, :],
                                                kr_all[:, tt, :],
                                                rstd_all[:, tt:tt + 1])
                # batched k-rope rotation
                krf = p0s.tile([128, NTT, DR], bf16, name="krf")
                x1 = kr_all[:, :, 0:DR:2]
                x2 = kr_all[:, :, 1:DR:2]
                ta = p0s.tile([128, NTT, DR // 2], f32, name="tak")
                tb = p0s.tile([128, NTT, DR // 2], f32, name="tbk")
                nc.vector.tensor_mul(ta[:], x1, cosk_sb[:])
                nc.vector.tensor_mul(tb[:], x2, sink_sb[:])
                nc.vector.tensor_sub(krf[:, :, 0:DR:2], ta[:], tb[:])
                nc.vector.tensor_mul(ta[:], x2, cosk_sb[:])
                nc.vector.tensor_mul(tb[:], x1, sink_sb[:])
                nc.vector.tensor_add(krf[:, :, 1:DR:2], ta[:], tb[:])
                for tt in range(NTT):
                    for rc in range(NKV):
                        tp = p0tp.tile([128, 128], bf16, tag="tp", name="tp")
                        nc.tensor.transpose(
                            tp[:], c_hat[:, tt, rc * 128:(rc + 1) * 128],
                            eye_sb[:])
                        nc.vector.tensor_copy(
                            kT_lat[:, rc, tt * 128:(tt + 1) * 128], tp[:])
                    tp = p0tp.tile([128, 128], bf16, tag="tp", name="tp")
                    nc.tensor.transpose(tp[0:64, :], krf[:, tt, :], eye_sb[:])
                    nc.vector.tensor_copy(kT_rope[:, tt * 128:(tt + 1) * 128],
                                          tp[0:64, :])

        # =============== phase 2: attention (all in T layouts) ===============
        if MAXPH >= 3:
            with tc.tile_pool(name="p2", bufs=2) as p2, \
                 tc.tile_pool(name="p2d", bufs=2) as p2d, \
                 tc.tile_pool(name="p2e", bufs=2) as p2e, \
                 tc.tile_pool(name="p2ps", bufs=3, space="PSUM") as p2ps, \
                 tc.tile_pool(name="olps", bufs=4, space="PSUM") as olps, \
                 tc.tile_pool(name="denps", bufs=1, space="PSUM") as denps:
                for h in range(NH):
                    qbw = p2.tile([128, NRC * DN], bf16, tag="qbw", name="qbw")
                    nc.gpsimd.dma_start(out=qbw[:], in_=qbn_blk[h])
                    wuk_h = p2.tile([128, NKV, 128], bf16, tag="wuk_h",
                                    name="wuk_h")
                    nc.gpsimd.dma_start(out=wuk_h[:], in_=wuk[h])
                    wuv_h = p2.tile([128, NKV, DV], bf16, tag="wuv_h",
                                    name="wuv_h")
                    nc.gpsimd.dma_start(out=wuv_h[:], in_=wuv[h])

                    # qnT[dn, q] = sum_rc qb_nope[rc].T @ qcT[rc]
                    qn_ps = p2ps.tile([128, RPC], f32, tag="sp", name="qnps")
                    for rc in range(NRC):
                        nc.tensor.matmul(qn_ps[:], qbw[:, rc * DN:(rc + 1) * DN],
                                         qcT[:, rc, :],
                                         start=(rc == 0), stop=(rc == NRC - 1))
                    qnT = p2d.tile([128, RPC], bf16, tag="qnT", name="qnT")
                    nc.scalar.copy(qnT[:], qn_ps[:])
                    # qT_lat[lat, q] = wuk.T @ qnT  (4 latent chunks)
                    qlat = p2d.tile([128, NKV, RPC], bf16, tag="qlat",
                                    name="qlat")
                    for rc in range(NKV):
                        lp = p2ps.tile([128, RPC], f32, tag="sp", name="latps")
                        nc.tensor.matmul(lp[:], wuk_h[:, rc, :], qnT[:],
                                         start=True, stop=True)
                        nc.scalar.copy(qlat[:, rc, :], lp[:])

                    ol = [olps.tile([128, RPC], f32, tag="ol", name="ol")
                          for _ in range(NKV)]
                    den = denps.tile([8, RPC], f32, tag="den", name="den")
                    eT = p2e.tile([128, NTT, RPC], bf16, tag="eT", name="eT")
                    for kt in range(NTT):
                        w0 = 16 * kt
                        sp = p2ps.tile([128, RPC], f32, tag="sp", name="sp")
                        for dc in range(NKV):
                            nc.tensor.matmul(
                                sp[:, w0:], kT_lat[:, dc, kt * 128:(kt + 1) * 128],
                                qlat[:, dc, w0:], start=(dc == 0), stop=False)
                        nc.tensor.matmul(
                            sp[:, w0:], kT_rope[:, kt * 128:(kt + 1) * 128],
                            qropeT[:, h, w0:], start=False, stop=True)
                        nc.scalar.activation(eT[:, kt, w0:], sp[:, w0:], AF.Exp)
                        nc.vector.tensor_mul(eT[:, kt, w0:w0 + 16],
                                             eT[:, kt, w0:w0 + 16], mask_sb[:])
                        nc.tensor.matmul(
                            den[:, w0:], c_hat[:, kt, 512:CAUG], eT[:, kt, w0:],
                            start=(kt == 0), stop=(kt == NTT - 1))
                    # 1/den on DVE overlaps the ol accumulation below
                    rinv = p2d.tile([1, RPC], bf16, tag="rinv", name="rinv")
                    with nc.allow_low_precision(
                            reason="1/den feeds bf16 matmul rhs anyway"):
                        nc.vector.reciprocal(rinv[:], den[0:1, :])
                    for kt in range(NTT):
                        w0 = 16 * kt
                        for rc in range(NKV):
                            nc.tensor.matmul(
                                ol[rc][:, w0:],
                                c_hat[:, kt, rc * 128:(rc + 1) * 128],
                                eT[:, kt, w0:],
                                start=(kt == 0), stop=(kt == NTT - 1))
                    rb_ps = p2ps.tile([128, RPC], f32, tag="sp", name="rbps")
                    nc.tensor.matmul(rb_ps[:], ones_sb[:], rinv[:],
                                     start=True, stop=True)
                    rb = p2d.tile([128, RPC], bf16, tag="rb", name="rb")
                    nc.scalar.copy(rb[:], rb_ps[:])
                    olT = p2d.tile([128, NKV, RPC], bf16, tag="olT", name="olT")
                    for rc in range(NKV):
                        nc.vector.tensor_copy(olT[:, rc, :], ol[rc][:])
                    ovp = p2ps.tile([128, RPC], f32, tag="sp", name="ovp")
                    for rc in range(NKV):
                        nc.tensor.matmul(ovp[:], wuv_h[:, rc, :], olT[:, rc, :],
                                         start=(rc == 0), stop=(rc == NKV - 1))
                    nc.vector.tensor_mul(o_vT[:, h, :], ovp[:], rb[:])

        # =============== phase 3: o_proj + residual + post-norm ===============
        if MAXPH >= 4:
            with tc.tile_pool(name="p3", bufs=4) as p3, \
                 tc.tile_pool(name="p3s", bufs=1) as p3s, \
                 tc.tile_pool(name="p3d", bufs=2) as p3d:
                hn = p3s.tile([128, NQT, H], bf16, name="hn")
                with tc.tile_pool(name="p3ps", bufs=2, space="PSUM") as p3ps:
                    op_ps = [p3ps.tile([128, H], f32, tag="opps", name="opps")
                             for _ in range(NQT)]
                    for hc in range(NH):
                        oww = p3.tile([128, H], bf16, tag="oww", name="oww")
                        nc.sync.dma_start(out=oww[:], in_=ow_blk[hc])
                        for qt in range(NQT):
                            for nn in range(4):
                                nc.tensor.matmul(
                                    op_ps[qt][:, nn * 512:(nn + 1) * 512],
                                    o_vT[:, hc, qt * 128:(qt + 1) * 128],
                                    oww[:, nn * 512:(nn + 1) * 512],
                                    start=(hc == 0), stop=(hc == NH - 1))
                    ssq2 = p3s.tile([128, NQT], f32, name="ssq2")
                    for qt in range(NQT):
                        nc.vector.tensor_add(x_rows_sb[:, qt, :],
                                             x_rows_sb[:, qt, :], op_ps[qt][:])
                    for qt in range(NQT):
                        scr = p3d.tile([128, H], bf16, tag="scr3", name="scr3")
                        nc.vector.scalar_tensor_tensor(
                            scr[:], x_rows_sb[:, qt, :], 1.0,
                            x_rows_sb[:, qt, :],
                            ALU.bypass, ALU.mult, accum_out=ssq2[:, qt:qt + 1])
                    nc.scalar.activation(ssq2[:], ssq2[:], AF.Ln,
                                         bias=eps_sb[:], scale=1.0 / H)
                    nc.scalar.activation(ssq2[:], ssq2[:], AF.Exp, scale=-0.5)
                    for qt in range(NQT):
                        nc.vector.tensor_scalar_mul(hn[:, qt, :],
                                                    x_rows_sb[:, qt, :],
                                                    ssq2[:, qt:qt + 1])
                with tc.tile_pool(name="p3tp", bufs=2, space="PSUM") as p3tp:
                    for qt in range(NQT):
                        for fc in range(NFC):
                            tp = p3tp.tile([128, 128], bf16, tag="tp",
                                           name="tp")
                            nc.tensor.transpose(
                                tp[:], hn[:, qt, fc * 128:(fc + 1) * 128],
                                eye_sb[:])
                            nc.vector.tensor_copy(
                                hnT[:, fc, qt * 128:(qt + 1) * 128], tp[:])

        # =============== phase 4: MLP ===============
        if MAXPH >= 5:
            with tc.tile_pool(name="p4", bufs=3) as p4, \
                 tc.tile_pool(name="p4ps", bufs=2, space="PSUM") as p4ps:
                GOFF = NFC * 128
                for it in range(NIT):
                    gw = p4.tile([128, 2 * NFC * 128], bf16, tag="gw", name="gw")
                    nc.sync.dma_start(out=gw[:], in_=gu_blk[it])
                    gp = p4ps.tile([128, RPC], f32, tag="gp", name="gp")
                    up = p4ps.tile([128, RPC], f32, tag="up", name="up")
                    for fc in range(NFC):
                        nc.tensor.matmul(gp[:], gw[:, fc * 128:(fc + 1) * 128],
                                         hnT[:, fc, :],
                                         start=(fc == 0), stop=(fc == NFC - 1))
                        nc.tensor.matmul(up[:],
                                         gw[:, GOFF + fc * 128:GOFF + (fc + 1) * 128],
                                         hnT[:, fc, :],
                                         start=(fc == 0), stop=(fc == NFC - 1))
                    gs = p4.tile([128, RPC], bf16, tag="gs", name="gs")
                    nc.scalar.activation(gs[:], gp[:], AF.Silu)
                    nc.vector.tensor_mul(act_all[:, it, :], gs[:], up[:])
            with tc.tile_pool(name="p4b", bufs=3) as p4b, \
                 tc.tile_pool(name="p4s", bufs=2) as p4s, \
                 tc.tile_pool(name="p4bps", bufs=2, space="PSUM") as p4bps:
                o_ps = [p4bps.tile([128, H], f32, tag="ops", name="ops")
                        for _ in range(NQT)]
                for jt in range(NIT // 2):
                    dw = p4b.tile([128, 2 * H], bf16, tag="dw", name="dw")
                    nc.sync.dma_start(out=dw[:], in_=dw_blk[jt])
                    for k in range(2):
                        it = 2 * jt + k
                        for qt in range(NQT):
                            for nn in range(4):
                                nc.tensor.matmul(
                                    o_ps[qt][:, nn * 512:(nn + 1) * 512],
                                    act_all[:, it, qt * 128:(qt + 1) * 128],
                                    dw[:, k * H + nn * 512:k * H + (nn + 1) * 512],
                                    start=(it == 0), stop=(it == NIT - 1))
                for qt in range(NQT):
                    fin = p4s.tile([128, H], f32, tag="fin", name="fin")
                    nc.vector.tensor_add(fin[:], x_rows_sb[:, qt, :],
                                         o_ps[qt][:])
                    nc.sync.dma_start(out=out_rows[qt], in_=fin[:])

        if MAXPH < 5:
            with tc.tile_pool(name="pex", bufs=2) as pex:
                for qt in range(NQT):
                    fin = pex.tile([128, H], f32, tag="finx", name="finx")
                    nc.vector.tensor_copy(fin[:], x_rows_sb[:, qt, :])
                    nc.sync.dma_start(out=out_rows[qt], in_=fin[:])
    nc.compile()
    return nc


def _host_prep(inputs):
    f32 = np.float32
    bf = bfloat16
    x = np.asarray(inputs["hidden_states"], f32)
    pos = np.asarray(inputs["positions"]).astype(f32)

    lnw_in = np.asarray(inputs["input_ln_w"], f32)
    q_a_w = np.asarray(inputs["q_a_w"], f32) * lnw_in[:, None]
    kv_a_w = np.asarray(inputs["kv_a_w"], f32) * lnw_in[:, None]
    q_b_w = (np.asarray(inputs["q_b_w"], f32)
             * np.asarray(inputs["q_a_ln_w"], f32)[:, None]) * SCALE
    kvln = np.asarray(inputs["kv_a_ln_w"], f32)
    w_uk = np.asarray(inputs["w_uk"], f32) * kvln[:, None, None]
    w_uv = np.asarray(inputs["w_uv"], f32) * kvln[:, None, None]
    o_w = np.asarray(inputs["o_w"], f32)
    pln = np.asarray(inputs["post_ln_w"], f32)
    gate_w = np.asarray(inputs["gate_w"], f32) * pln[:, None]
    up_w = np.asarray(inputs["up_w"], f32) * pln[:, None]
    down_w = np.asarray(inputs["down_w"], f32)

    xT = np.ascontiguousarray(x.T)
    inv_freq = 1.0 / (THETA ** (np.arange(0, DR, 2, dtype=f32) / DR))
    ang = pos[:, None] * inv_freq
    cos_t = np.cos(ang).astype(f32)
    sin_t = np.sin(ang).astype(f32)

    gu = np.zeros((2, IPAD, H), f32)
    gu[0, :INTER] = gate_w.T
    gu[1, :INTER] = up_w.T

    qb4 = q_b_w.astype(bf).reshape(NRC, 128, NH, DN + DR)

    rep = {
        "xT_blk": np.ascontiguousarray(
            xT.astype(bf).reshape(NFC, 128, NTT, 128).transpose(2, 1, 0, 3)
            .reshape(NTT // 2, 2, 128, NFC, 128).transpose(0, 2, 1, 3, 4)),
        "qa_blk": np.ascontiguousarray(
            q_a_w.astype(bf).reshape(NFC // 4, 4, 128, QLR)
            .transpose(0, 2, 1, 3).reshape(NFC // 4, 128, 4 * QLR)),
        # qbn_blk[h, p, rc*DN+d] = SCALE*q_b_w[rc*128+p, h*192+d], d < DN
        "qbn_blk": np.ascontiguousarray(
            qb4[:, :, :, :DN].transpose(2, 1, 0, 3).reshape(NH, 128, NRC * DN)),
        # qbr_blk[rc, p, h*64+j] = SCALE*q_b_w[rc*128+p, h*192+128+j]
        "qbr_blk": np.ascontiguousarray(
            qb4[:, :, :, DN:].reshape(NRC // 3, 3, 128, NH * DR)
            .transpose(0, 2, 1, 3).reshape(NRC // 3, 128, 3 * NH * DR)),
        "kva_blk": np.ascontiguousarray(
            kv_a_w.astype(bf).reshape(2, 8, 128, KVLR + DR)
            .transpose(0, 2, 1, 3).reshape(2, 128, 8 * (KVLR + DR))),
        "wuk": np.ascontiguousarray(
            w_uk.transpose(1, 2, 0).reshape(NH, 128, NKV, 128).astype(bf)),
        "wuv": np.ascontiguousarray(
            w_uv.transpose(1, 0, 2).reshape(NH, NKV, 128, DV)
            .transpose(0, 2, 1, 3).astype(bf)),
        "ow_blk": np.ascontiguousarray(o_w.astype(bf).reshape(NH, 128, H)),
        # gu_blk[it, hp, g*NFC*128 + fc*128 + ip] = gate/up[fc*128+hp, it*128+ip]
        "gu_blk": np.ascontiguousarray(
            gu.reshape(2, NIT, 128, NFC, 128).transpose(1, 4, 0, 3, 2)
            .reshape(NIT, 128, 2 * NFC * 128).astype(bf)),
        "dw_blk": np.ascontiguousarray(
            np.concatenate([down_w, np.zeros((IPAD - INTER, H), f32)], 0)
            .astype(bf).reshape(NIT // 2, 2, 128, H).transpose(0, 2, 1, 3)
            .reshape(NIT // 2, 128, 2 * H)),
        "cosk": np.ascontiguousarray(
            cos_t.reshape(NTT, 128, DR // 2).transpose(1, 0, 2)),
        "sink": np.ascontiguousarray(
            sin_t.reshape(NTT, 128, DR // 2).transpose(1, 0, 2)),
        "eye": np.eye(128, dtype=bf),
        "ones_row": np.ones((1, 128), bf),
    }

    per_core = []
    kpos = np.arange(128)
    qpos = np.arange(16)
    for c in range(NCORES):
        rows = np.arange(c, T, NCORES)
        m = dict(rep)
        m["x_rows"] = np.ascontiguousarray(x[rows].reshape(NQT, 128, H))
        m["xTc"] = np.ascontiguousarray(
            xT[:, rows].astype(bf).reshape(NFC, 128, RPC)
            .transpose(1, 0, 2).reshape(128, NFC * RPC))
        cq = np.broadcast_to(
            cos_t[rows].reshape(NQT, 128, 1, DR // 2),
            (NQT, 128, NH, DR // 2))
        sq = np.broadcast_to(
            sin_t[rows].reshape(NQT, 128, 1, DR // 2),
            (NQT, 128, NH, DR // 2))
        m["cosq16"] = np.ascontiguousarray(
            cq.reshape(NQT, 128, NH * DR // 2).astype(bf))
        m["sinq16"] = np.ascontiguousarray(
            sq.reshape(NQT, 128, NH * DR // 2).astype(bf))
        # mask16[j, i] = key j attends-allowed for query row (16*kt + i):
        # key pos 128*kt + j <= query pos 8*(16*kt+i) + c  <=>  j <= 8*i + c
        m["mask16"] = np.ascontiguousarray(
            (kpos[:, None] <= 8 * qpos[None, :] + c).astype(bf))
        per_core.append(m)
    return per_core


def kernel(**inputs):
    from concourse import bass_utils

    if "nc" not in _CACHE:
        _CACHE["nc"] = _build_module()
    nc = _CACHE["nc"]

    import os
    in_maps = _host_prep(inputs)
    trace = bool(os.environ.get("BASS_KERNEL_TRACE"))
    res = bass_utils.run_bass_kernel_spmd(nc, in_maps,
                                          core_ids=list(range(NCORES)),
                                          trace=trace)
    if trace and res.exec_time_ns is not None:
        print(f"HW exec time: {res.exec_time_ns} ns")
        _CACHE["last_result"] = res
    out = np.zeros((T, H), np.float32)
    for c in range(NCORES):
        rows = np.arange(c, T, NCORES)
        out[rows] = res.results[c]["out_rows"].reshape(RPC, H)
    return out
